# revision 5
# baseline (speedup 1.0000x reference)
"""Trainium2 Bass kernel: 8-head transformer encoder layer (B=8, S=1024,
D=300, Dh=512, H=8), data-parallel over batch across 8 NeuronCores.

Per core (one batch element):
  qT/kT = Wp @ x^T  (heads contiguous via host-side weight-row permute)
  v     = x @ Wp^T
  per head: e = q k^T (PSUM) -> bn_stats var -> c = gamma*sqrt(D)/sd
            p = exp(c*e) (ACT, accum row-sum r) -> p *= 1/r (GPSIMD)
            pT via PE transpose -> heads^T = v^T-chunks @ pT (PSUM acc)
  x1 = a @ WO ; x2 = LN(x1 + x) ; x2T via PE transpose
  h1T = relu(W1^T-form @ x2T + b1) ; h2 = h1T-chunks @ W2
  out = LN(h2 + b2 + x2)
"""

import math

import numpy as np

import concourse.bass as bass
import concourse.tile as tile
from concourse import bacc, mybir
from concourse.bass_utils import run_bass_kernel_spmd
from concourse.masks import make_identity

F32 = mybir.dt.float32
AF = mybir.ActivationFunctionType

B, S, D, DH, H, DHD = 8, 1024, 300, 512, 8, 64
DF = 4 * D  # 1200
EPS = 1e-8
NCORES = 8

J_CHUNKS = [(0, 128), (128, 128), (256, 44)]  # D=300 partition chunks
M_CHUNKS = [(i * 128, min(128, DF - i * 128)) for i in range(10)]  # DF=1200
N_ST = S // 128  # 8 s-tiles
N_SH = S // 512  # 2 s-halves

TRACE = False
_cache = {}
_last_results = None


def _build_nc(dbg=False):
    nc = bacc.Bacc("TRN2", debug=False)

    xd = nc.dram_tensor("x", [S, D], F32, kind="ExternalInput").ap()
    xtd = nc.dram_tensor("xt", [D, S], F32, kind="ExternalInput").ap()
    wqd = nc.dram_tensor("wq", [D, DH], F32, kind="ExternalInput").ap()
    wkd = nc.dram_tensor("wk", [D, DH], F32, kind="ExternalInput").ap()
    wvd = nc.dram_tensor("wv", [D, DH], F32, kind="ExternalInput").ap()
    wod = nc.dram_tensor("wo", [DH, D], F32, kind="ExternalInput").ap()
    w1d = nc.dram_tensor("w1", [D, DF], F32, kind="ExternalInput").ap()
    w2d = nc.dram_tensor("w2", [DF, D], F32, kind="ExternalInput").ap()
    fb1d = nc.dram_tensor("fb1", [1280, 1], F32, kind="ExternalInput").ap()
    fb2d = nc.dram_tensor("fb2", [D], F32, kind="ExternalInput").ap()
    gad = nc.dram_tensor("ga", [H, 1], F32, kind="ExternalInput").ap()
    lnd = nc.dram_tensor("ln", [4, 1], F32, kind="ExternalInput").ap()
    outd = nc.dram_tensor("out", [S, D], F32, kind="ExternalOutput").ap()
    if dbg:
        dqT = nc.dram_tensor("dqT", [DH, S], F32, kind="ExternalOutput").ap()
        dkT = nc.dram_tensor("dkT", [DH, S], F32, kind="ExternalOutput").ap()
        dv = nc.dram_tensor("dv", [S, DH], F32, kind="ExternalOutput").ap()
        dp0 = nc.dram_tensor("dp0", [S, S], F32, kind="ExternalOutput").ap()
        daT = nc.dram_tensor("daT", [DH, S], F32, kind="ExternalOutput").ap()
        dx2 = nc.dram_tensor("dx2", [S, D], F32, kind="ExternalOutput").ap()
        dh1 = nc.dram_tensor("dh1", [DF, S], F32, kind="ExternalOutput").ap()
        dr = nc.dram_tensor("dr", [H, S], F32, kind="ExternalOutput").ap()
        dc = nc.dram_tensor("dc", [H, S], F32, kind="ExternalOutput").ap()
        dmv = nc.dram_tensor("dmv", [H, S, 2], F32, kind="ExternalOutput").ap()

    with tile.TileContext(nc) as tc:
        with (
            tc.tile_pool(name="wts", bufs=1) as wts,
            tc.tile_pool(name="work", bufs=1) as work,
            tc.tile_pool(name="sm", bufs=8) as sm,
            tc.tile_pool(name="ps", bufs=1, space="PSUM") as ps,
        ):
            # ---------------- constant / weight loads ----------------
            ident = wts.tile([128, 128], F32, tag="ident")
            make_identity(nc, ident)

            eps_a = wts.tile([128, 1], F32, tag="eps_a")  # D*EPS (score LN)
            nc.vector.memset(eps_a, D * EPS)
            eps_l = wts.tile([128, 1], F32, tag="eps_l")  # EPS (x LNs)
            nc.vector.memset(eps_l, EPS)

            def bcast_load(src_ap, shape, tag):
                t = wts.tile(shape, F32, tag=tag)
                nc.sync.dma_start(out=t, in_=src_ap.to_broadcast(shape))
                return t

            ga_bc = [bcast_load(gad[h : h + 1, :], [128, 1], f"ga{h}") for h in range(H)]
            g1_bc = bcast_load(lnd[0:1, :], [128, 1], "g1")
            b1_bc = bcast_load(lnd[1:2, :], [128, 1], "b1")
            g2_bc = bcast_load(lnd[2:3, :], [128, 1], "g2")
            b2_bc = bcast_load(lnd[3:4, :], [128, 1], "b2")
            fb2_bc = wts.tile([128, D], F32, tag="fb2")
            nc.sync.dma_start(
                out=fb2_bc,
                in_=bass.AP(tensor=fb2d.tensor, offset=fb2d.offset, ap=[[0, 128]] + list(fb2d.ap)),
            )
            fb1_sb = []
            for mt, (m0, msz) in enumerate(M_CHUNKS):
                t = wts.tile([128, 1], F32, tag=f"fb1_{mt}")
                nc.sync.dma_start(out=t[:msz, :], in_=fb1d[m0 : m0 + msz, :])
                fb1_sb.append(t)

            # x natural: [128, 8, 300] (partition = s % 128)
            x_sb = wts.tile([128, N_ST, D], F32, tag="x")
            nc.sync.dma_start(out=x_sb, in_=xd.rearrange("(n p) d -> p n d", p=128))

            def chunked_load(src, width, tag):
                tiles = []
                for jc, (j0, jn) in enumerate(J_CHUNKS):
                    t = wts.tile([128, width], F32, tag=f"{tag}{jc}")
                    nc.sync.dma_start(out=t[:jn, :], in_=src[j0 : j0 + jn, :])
                    tiles.append(t)
                return tiles

            xt_sb = chunked_load(xtd, S, "xt")    # [300, 1024] in 3 chunks
            wq_sb = chunked_load(wqd, DH, "wq")   # [300, 512]
            wk_sb = chunked_load(wkd, DH, "wk")
            wv_sb = chunked_load(wvd, DH, "wv")
            w1_sb = chunked_load(w1d, DF, "w1")   # [300, 1200]

            wo_sb = []
            for it in range(4):
                t = wts.tile([128, D], F32, tag=f"wo{it}")
                nc.sync.dma_start(out=t, in_=wod[it * 128 : (it + 1) * 128, :])
                wo_sb.append(t)
            w2_sb = []
            for mt, (m0, msz) in enumerate(M_CHUNKS):
                t = wts.tile([128, D], F32, tag=f"w2_{mt}")
                nc.sync.dma_start(out=t[:msz, :], in_=w2d[m0 : m0 + msz, :])
                w2_sb.append(t)

            # ---------------- phase 1: projections ----------------
            # qT/kT [Dh, S] as 4 x [128, 1024];  v [S, Dh] as 8 x [128, 512]
            qT = [work.tile([128, S], F32, tag="big4k", bufs=14, name=f"qT{i}") for i in range(4)]
            kT = [work.tile([128, S], F32, tag="big4k", bufs=14, name=f"kT{i}") for i in range(4)]
            v_sb = [work.tile([128, DH], F32, tag="v2k", bufs=9, name=f"v{i}") for i in range(N_ST)]

            for dst, w in ((qT, wq_sb), (kT, wk_sb)):
                for dt in range(4):
                    for sh in range(N_SH):
                        pp = ps.tile([128, 512], F32, tag="e", bufs=2)
                        for jc, (j0, jn) in enumerate(J_CHUNKS):
                            nc.tensor.matmul(
                                pp,
                                lhsT=w[jc][:jn, dt * 128 : (dt + 1) * 128],
                                rhs=xt_sb[jc][:jn, sh * 512 : (sh + 1) * 512],
                                start=(jc == 0),
                                stop=(jc == 2),
                            )
                        nc.scalar.copy(out=dst[dt][:, sh * 512 : (sh + 1) * 512], in_=pp)
            for st in range(N_ST):
                pp = ps.tile([128, 512], F32, tag="e", bufs=2)
                for jc, (j0, jn) in enumerate(J_CHUNKS):
                    nc.tensor.matmul(
                        pp,
                        lhsT=xt_sb[jc][:jn, st * 128 : (st + 1) * 128],
                        rhs=wv_sb[jc][:jn, :],
                        start=(jc == 0),
                        stop=(jc == 2),
                    )
                nc.scalar.copy(out=v_sb[st], in_=pp)

            if dbg:
                for i in range(4):
                    nc.sync.dma_start(out=dqT[i * 128 : (i + 1) * 128, :], in_=qT[i])
                    nc.sync.dma_start(out=dkT[i * 128 : (i + 1) * 128, :], in_=kT[i])
                for i in range(N_ST):
                    nc.sync.dma_start(out=dv[i * 128 : (i + 1) * 128, :], in_=v_sb[i])

            # ---------------- phase 2: attention ----------------
            aT = [work.tile([128, S], F32, tag="big4k", bufs=14, name=f"aT{i}") for i in range(4)]
            SCORR = float(S) / float(S - 1)

            for h in range(H):
                qt_t, hp = qT[h // 2], (h % 2) * 64
                kt_t = kT[h // 2]
                for sh in range(N_SH):
                    pT = work.tile([128, 8, 512], F32, tag="pt16k", bufs=2)
                    for st4 in range(4):
                        st = sh * 4 + st4
                        e_ps = ps.tile([128, S], F32, tag="e", bufs=2)
                        for th in range(N_SH):
                            nc.tensor.matmul(
                                e_ps[:, th * 512 : (th + 1) * 512],
                                lhsT=qt_t[hp : hp + 64, st * 128 : (st + 1) * 128],
                                rhs=kt_t[hp : hp + 64, th * 512 : (th + 1) * 512],
                                start=True,
                                stop=True,
                            )
                        stats = sm.tile([128, 2, 6], F32, tag="stats", bufs=4)
                        nc.vector.bn_stats(out=stats[:, 0, :], in_=e_ps[:, 0:512])
                        nc.vector.bn_stats(out=stats[:, 1, :], in_=e_ps[:, 512:1024])
                        mv = sm.tile([128, 2], F32, tag="mv", bufs=4)
                        nc.vector.bn_aggr(out=mv, in_=stats)
                        # sd = sqrt(var*S/(S-1) + D*eps); c = ga*sqrt(D)/sd
                        sd = sm.tile([128, 1], F32, tag="sd", bufs=4)
                        nc.scalar.activation(
                            out=sd, in_=mv[:, 1:2], func=AF.Sqrt, bias=eps_a, scale=SCORR
                        )
                        rstd = sm.tile([128, 1], F32, tag="rstd", bufs=4)
                        nc.vector.reciprocal(out=rstd, in_=sd)
                        c = sm.tile([128, 1], F32, tag="c", bufs=4)
                        nc.vector.tensor_mul(c, rstd, ga_bc[h])
                        # p = exp(c*e), r = row-sum(p)
                        p_sb = work.tile([128, S], F32, tag="big4k", bufs=14)
                        r = sm.tile([128, 1], F32, tag="r", bufs=4)
                        nc.scalar.activation(
                            out=p_sb, in_=e_ps, func=AF.Exp, bias=0.0, scale=c, accum_out=r
                        )
                        if dbg:
                            nc.sync.dma_start(out=dr[h, st * 128 : (st + 1) * 128], in_=r)
                            nc.sync.dma_start(out=dc[h, st * 128 : (st + 1) * 128], in_=c)
                            nc.sync.dma_start(out=dmv[h, st * 128 : (st + 1) * 128, :], in_=mv)
                        rinv = sm.tile([128, 1], F32, tag="rinv", bufs=4)
                        nc.vector.reciprocal(out=rinv, in_=r)
                        nc.gpsimd.tensor_scalar_mul(p_sb, p_sb, rinv)
                        if dbg and h == 0:
                            nc.sync.dma_start(
                                out=dp0[st * 128 : (st + 1) * 128, :], in_=p_sb
                            )
                        # transpose p -> pT[:, tj, st4*128:...]
                        for half in range(2):
                            pt_ps = ps.tile([128, 4, 128], F32, tag="pt", bufs=2)
                            for i in range(4):
                                tj = half * 4 + i
                                nc.tensor.transpose(
                                    pt_ps[:, i, :],
                                    p_sb[:, tj * 128 : (tj + 1) * 128],
                                    ident,
                                )
                            dst = pT[:, half * 4 : half * 4 + 4, st4 * 128 : (st4 + 1) * 128]
                            if half == 0:
                                nc.scalar.copy(out=dst, in_=pt_ps)
                            else:
                                nc.vector.tensor_copy(out=dst, in_=pt_ps)
                    # AV for this half: heads^T [64, 512]
                    av_ps = ps.tile([64, 512], F32, tag="acc", bufs=2)
                    for tj in range(8):
                        nc.tensor.matmul(
                            av_ps,
                            lhsT=v_sb[tj][:, h * 64 : (h + 1) * 64],
                            rhs=pT[:, tj, :],
                            start=(tj == 0),
                            stop=(tj == 7),
                        )
                    nc.scalar.copy(
                        out=aT[h // 2][hp : hp + 64, sh * 512 : (sh + 1) * 512], in_=av_ps
                    )

            if dbg:
                for i in range(4):
                    nc.sync.dma_start(out=daT[i * 128 : (i + 1) * 128, :], in_=aT[i])

            # ---------------- phase 3: WO + residual + LN1 ----------------
            x2_sb = [work.tile([128, D], F32, tag="v2k", bufs=9, name=f"x2_{i}") for i in range(N_ST)]
            x2T = [work.tile([128, S], F32, tag="big4k", bufs=14, name=f"x2T{i}") for i in range(3)]
            LCORR = float(D) / float(D - 1)

            def layer_norm(dst, src_ps, res_tiles, g_bc, b_bc, extra=None):
                # dst = LN(src_ps + residuals) * g + b   (src_ps in PSUM)
                xr = sm.tile([128, D], F32, tag="xr", bufs=4)
                nc.vector.tensor_add(xr, src_ps, res_tiles[0])
                for rt in res_tiles[1:]:
                    nc.vector.tensor_add(xr, xr, rt)
                stats = sm.tile([128, 6], F32, tag="lstats", bufs=4)
                nc.vector.bn_stats(out=stats, in_=xr)
                mv = sm.tile([128, 2], F32, tag="lmv", bufs=4)
                nc.vector.bn_aggr(out=mv, in_=stats)
                sd = sm.tile([128, 1], F32, tag="lsd", bufs=4)
                nc.scalar.activation(
                    out=sd, in_=mv[:, 1:2], func=AF.Sqrt, bias=eps_l, scale=LCORR
                )
                rstd = sm.tile([128, 1], F32, tag="lrstd", bufs=4)
                nc.vector.reciprocal(out=rstd, in_=sd)
                grstd = sm.tile([128, 1], F32, tag="lgr", bufs=4)
                nc.vector.tensor_mul(grstd, rstd, g_bc)
                nc.vector.tensor_scalar(
                    out=dst,
                    in0=xr,
                    scalar1=mv[:, 0:1],
                    scalar2=grstd,
                    op0=mybir.AluOpType.subtract,
                    op1=mybir.AluOpType.mult,
                )
                nc.vector.tensor_scalar_add(dst, dst, b_bc)

            for st in range(N_ST):
                x1_ps = ps.tile([128, D], F32, tag="acc", bufs=2)
                for it in range(4):
                    nc.tensor.matmul(
                        x1_ps,
                        lhsT=aT[it][:, st * 128 : (st + 1) * 128],
                        rhs=wo_sb[it],
                        start=(it == 0),
                        stop=(it == 3),
                    )
                layer_norm(x2_sb[st], x1_ps, [x_sb[:, st, :]], g1_bc, b1_bc)
                # transpose x2[st] -> x2T chunks
                xt_ps = ps.tile([128, 4, 128], F32, tag="pt", bufs=2)
                for jc, (j0, jn) in enumerate(J_CHUNKS):
                    nc.tensor.transpose(
                        xt_ps[:jn, jc, :], x2_sb[st][:, j0 : j0 + jn], ident
                    )
                for jc, (j0, jn) in enumerate(J_CHUNKS):
                    nc.vector.tensor_copy(
                        out=x2T[jc][:jn, st * 128 : (st + 1) * 128],
                        in_=xt_ps[:jn, jc, :],
                    )

            # ---------------- phase 4: FFN + LN2 ----------------
            h1T = [work.tile([128, S], F32, tag="big4k", bufs=14, name=f"h1T{i}") for i in range(10)]
            for mt, (m0, msz) in enumerate(M_CHUNKS):
                for sh in range(N_SH):
                    h1_ps = ps.tile([128, 512], F32, tag="acc", bufs=2)
                    for jc, (j0, jn) in enumerate(J_CHUNKS):
                        nc.tensor.matmul(
                            h1_ps[:msz, :],
                            lhsT=w1_sb[jc][:jn, m0 : m0 + msz],
                            rhs=x2T[jc][:jn, sh * 512 : (sh + 1) * 512],
                            start=(jc == 0),
                            stop=(jc == 2),
                        )
                    nc.scalar.activation(
                        out=h1T[mt][:msz, sh * 512 : (sh + 1) * 512],
                        in_=h1_ps[:msz, :],
                        func=AF.Relu,
                        bias=fb1_sb[mt][:msz, :],
                        scale=1.0,
                    )
            if dbg:
                for i in range(N_ST):
                    nc.sync.dma_start(out=dx2[i * 128 : (i + 1) * 128, :], in_=x2_sb[i])
                for mt, (m0, msz) in enumerate(M_CHUNKS):
                    nc.sync.dma_start(out=dh1[m0 : m0 + msz, :], in_=h1T[mt][:msz, :])
            for st in range(N_ST):
                h2_ps = ps.tile([128, D], F32, tag="acc", bufs=2)
                for mt, (m0, msz) in enumerate(M_CHUNKS):
                    nc.tensor.matmul(
                        h2_ps,
                        lhsT=h1T[mt][:msz, st * 128 : (st + 1) * 128],
                        rhs=w2_sb[mt][:msz, :],
                        start=(mt == 0),
                        stop=(mt == 9),
                    )
                o_sb = sm.tile([128, D], F32, tag="o", bufs=4)
                layer_norm(o_sb, h2_ps, [fb2_bc, x2_sb[st]], g2_bc, b2_bc)
                nc.sync.dma_start(out=outd[st * 128 : (st + 1) * 128, :], in_=o_sb)

    nc.compile()
    return nc


def _get_nc():
    if "nc" not in _cache:
        _cache["nc"] = _build_nc()
    return _cache["nc"]


def kernel(x, WQ, WK, WV, WO, W1, b1, W2, b2, gamma_a, beta_a,
           gamma1, beta1, gamma2, beta2):
    global _last_results
    f = np.float32
    x = np.asarray(x, f)

    def perm(W):
        # head h -> contiguous rows [h*64, (h+1)*64)
        return np.asarray(W, f).reshape(DHD, H, D).transpose(1, 0, 2).reshape(DH, D)

    wq_t = np.ascontiguousarray(perm(WQ).T)
    wk_t = np.ascontiguousarray(perm(WK).T)
    wv_t = np.ascontiguousarray(perm(WV).T)
    wo = np.ascontiguousarray(np.asarray(WO, f))
    w1 = np.ascontiguousarray(np.asarray(W1, f))
    w2 = np.ascontiguousarray(np.asarray(W2, f))
    fb1 = np.zeros((1280, 1), f)
    fb1[:DF, 0] = np.asarray(b1, f)
    fb2 = np.ascontiguousarray(np.asarray(b2, f))
    # beta_a drops out of softmax (per-row constant shift); the 1/sqrt(D)
    # score scale cancels inside the score LayerNorm: softmax(g*LN(e/sqrt(D)))
    # == softmax(g/sqrt(var(e) + D*eps) * e), so gamma is used unscaled and
    # D*eps replaces eps on-device.
    ga = np.ascontiguousarray(np.asarray(gamma_a, f).reshape(H, 1))
    ln = np.array(
        [np.asarray(gamma1, f), np.asarray(beta1, f),
         np.asarray(gamma2, f), np.asarray(beta2, f)], f
    ).reshape(4, 1)

    shared = {"wq": wq_t, "wk": wk_t, "wv": wv_t, "wo": wo, "w1": w1, "w2": w2,
              "fb1": fb1, "fb2": fb2, "ga": ga, "ln": ln}
    in_maps = []
    for b in range(B):
        xb = np.ascontiguousarray(x[b])
        in_maps.append({"x": xb, "xt": np.ascontiguousarray(xb.T), **shared})

    nc = _get_nc()
    res = run_bass_kernel_spmd(nc, in_maps, core_ids=list(range(NCORES)), trace=TRACE)
    _last_results = res
    return np.stack([res.results[b]["out"] for b in range(B)], axis=0)


# revision 8
# speedup vs baseline: 1.6449x; 1.6449x over previous
"""Trainium2 Bass kernel: 8-head transformer encoder layer (B=8, S=1024,
D=300, Dh=512, H=8), data-parallel over batch across 8 NeuronCores.

Per core (one batch element):
  qT/kT = Wp @ x^T  (heads contiguous via host-side weight-row permute)
  v     = x @ Wp^T
  per head: e = q k^T (PSUM) -> bn_stats var -> c = gamma*sqrt(D)/sd
            p = exp(c*e) (ACT, accum row-sum r) -> p *= 1/r (GPSIMD)
            pT via PE transpose -> heads^T = v^T-chunks @ pT (PSUM acc)
  x1 = a @ WO ; x2 = LN(x1 + x) ; x2T via PE transpose
  h1T = relu(W1^T-form @ x2T + b1) ; h2 = h1T-chunks @ W2
  out = LN(h2 + b2 + x2)
"""

import math

import numpy as np

import concourse.bass as bass
import concourse.tile as tile
from concourse import bacc, mybir
from concourse.bass_utils import run_bass_kernel_spmd
from concourse.masks import make_identity

F32 = mybir.dt.float32
AF = mybir.ActivationFunctionType

B, S, D, DH, H, DHD = 8, 1024, 300, 512, 8, 64
DF = 4 * D  # 1200
EPS = 1e-8
NCORES = 8

J_CHUNKS = [(0, 128), (128, 128), (256, 44)]  # D=300 partition chunks
M_CHUNKS = [(i * 128, min(128, DF - i * 128)) for i in range(10)]  # DF=1200
N_ST = S // 128  # 8 s-tiles
N_SH = S // 512  # 2 s-halves

TRACE = False
_cache = {}
_last_results = None


def _build_nc(dbg=False):
    nc = bacc.Bacc("TRN2", debug=False)

    xd = nc.dram_tensor("x", [S, D], F32, kind="ExternalInput").ap()
    xtd = nc.dram_tensor("xt", [D, S], F32, kind="ExternalInput").ap()
    wqd = nc.dram_tensor("wq", [D, DH], F32, kind="ExternalInput").ap()
    wkd = nc.dram_tensor("wk", [D, DH], F32, kind="ExternalInput").ap()
    wvd = nc.dram_tensor("wv", [D, DH], F32, kind="ExternalInput").ap()
    wod = nc.dram_tensor("wo", [DH, D], F32, kind="ExternalInput").ap()
    w1d = nc.dram_tensor("w1", [D, DF], F32, kind="ExternalInput").ap()
    w2d = nc.dram_tensor("w2", [DF, D], F32, kind="ExternalInput").ap()
    fb1d = nc.dram_tensor("fb1", [1280, 1], F32, kind="ExternalInput").ap()
    fb2d = nc.dram_tensor("fb2", [D], F32, kind="ExternalInput").ap()
    gad = nc.dram_tensor("ga", [H, 1], F32, kind="ExternalInput").ap()
    lnd = nc.dram_tensor("ln", [4, 1], F32, kind="ExternalInput").ap()
    outd = nc.dram_tensor("out", [S, D], F32, kind="ExternalOutput").ap()
    if dbg:
        dqT = nc.dram_tensor("dqT", [DH, S], F32, kind="ExternalOutput").ap()
        dkT = nc.dram_tensor("dkT", [DH, S], F32, kind="ExternalOutput").ap()
        dv = nc.dram_tensor("dv", [S, DH], F32, kind="ExternalOutput").ap()
        dp0 = nc.dram_tensor("dp0", [S, S], F32, kind="ExternalOutput").ap()
        daT = nc.dram_tensor("daT", [DH, S], F32, kind="ExternalOutput").ap()
        dx2 = nc.dram_tensor("dx2", [S, D], F32, kind="ExternalOutput").ap()
        dh1 = nc.dram_tensor("dh1", [DF, S], F32, kind="ExternalOutput").ap()
        dr = nc.dram_tensor("dr", [H, S], F32, kind="ExternalOutput").ap()
        dc = nc.dram_tensor("dc", [H, S], F32, kind="ExternalOutput").ap()
        dmv = nc.dram_tensor("dmv", [H, S, 2], F32, kind="ExternalOutput").ap()

    with tile.TileContext(nc) as tc:
        with (
            tc.tile_pool(name="wts", bufs=1) as wts,
            tc.tile_pool(name="work", bufs=1) as work,
            tc.tile_pool(name="sm", bufs=8) as sm,
            tc.tile_pool(name="ps", bufs=1, space="PSUM") as ps,
        ):
            # ---------------- constant / weight loads ----------------
            ident = wts.tile([128, 128], F32, tag="ident")
            make_identity(nc, ident)

            ones1 = wts.tile([1, 128], F32, tag="ones1")
            nc.vector.memset(ones1, 1.0)

            eps_a = wts.tile([128, 1], F32, tag="eps_a")  # D*EPS (score LN)
            nc.vector.memset(eps_a, D * EPS)
            eps_l = wts.tile([128, 1], F32, tag="eps_l")  # EPS (x LNs)
            nc.vector.memset(eps_l, EPS)

            def bcast_load(src_ap, shape, tag):
                t = wts.tile(shape, F32, tag=tag)
                nc.sync.dma_start(out=t, in_=src_ap.to_broadcast(shape))
                return t

            ga_bc = [bcast_load(gad[h : h + 1, :], [128, 1], f"ga{h}") for h in range(H)]
            g1_bc = bcast_load(lnd[0:1, :], [128, 1], "g1")
            b1_bc = bcast_load(lnd[1:2, :], [128, 1], "b1")
            g2_bc = bcast_load(lnd[2:3, :], [128, 1], "g2")
            b2_bc = bcast_load(lnd[3:4, :], [128, 1], "b2")
            fb2_bc = wts.tile([128, D], F32, tag="fb2")
            nc.sync.dma_start(
                out=fb2_bc,
                in_=bass.AP(tensor=fb2d.tensor, offset=fb2d.offset, ap=[[0, 128]] + list(fb2d.ap)),
            )
            fb1_sb = []
            for mt, (m0, msz) in enumerate(M_CHUNKS):
                t = wts.tile([128, 1], F32, tag=f"fb1_{mt}")
                nc.sync.dma_start(out=t[:msz, :], in_=fb1d[m0 : m0 + msz, :])
                fb1_sb.append(t)

            # x natural: [128, 8, 300] (partition = s % 128)
            x_sb = wts.tile([128, N_ST, D], F32, tag="x")
            nc.sync.dma_start(out=x_sb, in_=xd.rearrange("(n p) d -> p n d", p=128))

            def chunked_load(src, width, tag):
                tiles = []
                for jc, (j0, jn) in enumerate(J_CHUNKS):
                    t = wts.tile([128, width], F32, tag=f"{tag}{jc}")
                    nc.sync.dma_start(out=t[:jn, :], in_=src[j0 : j0 + jn, :])
                    tiles.append(t)
                return tiles

            xt_sb = chunked_load(xtd, S, "xt")    # [300, 1024] in 3 chunks
            wq_sb = chunked_load(wqd, DH, "wq")   # [300, 512]
            wk_sb = chunked_load(wkd, DH, "wk")
            wv_sb = chunked_load(wvd, DH, "wv")
            w1_sb = chunked_load(w1d, DF, "w1")   # [300, 1200]

            wo_sb = []
            for it in range(4):
                t = wts.tile([128, D], F32, tag=f"wo{it}")
                nc.sync.dma_start(out=t, in_=wod[it * 128 : (it + 1) * 128, :])
                wo_sb.append(t)
            w2_sb = []
            for mt, (m0, msz) in enumerate(M_CHUNKS):
                t = wts.tile([128, D], F32, tag=f"w2_{mt}")
                nc.sync.dma_start(out=t[:msz, :], in_=w2d[m0 : m0 + msz, :])
                w2_sb.append(t)

            # ---------------- phase 1: projections ----------------
            # qT/kT [Dh, S] as 4 x [128, 1024];  v [S, Dh] as 8 x [128, 512]
            qT = [work.tile([128, S], F32, tag="big4k", bufs=14, name=f"qT{i}") for i in range(4)]
            kT = [work.tile([128, S], F32, tag="big4k", bufs=14, name=f"kT{i}") for i in range(4)]
            v_sb = [work.tile([128, H, DHD + 1], F32, tag="v2k", bufs=9, name=f"v{i}") for i in range(N_ST)]

            for dst, w in ((qT, wq_sb), (kT, wk_sb)):
                for dt in range(4):
                    for sh in range(N_SH):
                        pp = ps.tile([128, 512], F32, tag="e", bufs=2)
                        for jc, (j0, jn) in enumerate(J_CHUNKS):
                            nc.tensor.matmul(
                                pp,
                                lhsT=w[jc][:jn, dt * 128 : (dt + 1) * 128],
                                rhs=xt_sb[jc][:jn, sh * 512 : (sh + 1) * 512],
                                start=(jc == 0),
                                stop=(jc == 2),
                            )
                        nc.vector.tensor_copy(out=dst[dt][:, sh * 512 : (sh + 1) * 512], in_=pp)
            for st in range(N_ST):
                pp = ps.tile([128, 512], F32, tag="e", bufs=2)
                for jc, (j0, jn) in enumerate(J_CHUNKS):
                    nc.tensor.matmul(
                        pp,
                        lhsT=xt_sb[jc][:jn, st * 128 : (st + 1) * 128],
                        rhs=wv_sb[jc][:jn, :],
                        start=(jc == 0),
                        stop=(jc == 2),
                    )
                nc.vector.tensor_copy(
                    out=v_sb[st][:, :, 0:DHD],
                    in_=pp.rearrange("p (h d) -> p h d", h=H),
                )
                nc.vector.memset(v_sb[st][:, :, DHD : DHD + 1], 1.0)

            if dbg:
                for i in range(4):
                    nc.sync.dma_start(out=dqT[i * 128 : (i + 1) * 128, :], in_=qT[i])
                    nc.sync.dma_start(out=dkT[i * 128 : (i + 1) * 128, :], in_=kT[i])
                for i in range(N_ST):
                    nc.sync.dma_start(out=dv[i * 128 : (i + 1) * 128, :], in_=v_sb[i][:, :, 0:DHD])

            # ---------------- phase 2: attention ----------------
            aT = [work.tile([128, S], F32, tag="big4k", bufs=14, name=f"aT{i}") for i in range(4)]
            SCORR = float(S) / float(S - 1)

            for h in range(H):
                qt_t, hp = qT[h // 2], (h % 2) * 64
                kt_t = kT[h // 2]
                for sh in range(N_SH):
                    pT = work.tile([128, 8, 512], F32, tag="pt16k", bufs=2)
                    for pair in range(2):
                        e_pair = []
                        mv2 = sm.tile([128, 2, 2], F32, tag="mv", bufs=4)
                        for i in range(2):
                            st = sh * 4 + pair * 2 + i
                            e_ps = ps.tile([128, S], F32, tag="e", bufs=2)
                            e_pair.append((i, st, e_ps))
                            for th in range(N_SH):
                                nc.tensor.matmul(
                                    e_ps[:, th * 512 : (th + 1) * 512],
                                    lhsT=qt_t[hp : hp + 64, st * 128 : (st + 1) * 128],
                                    rhs=kt_t[hp : hp + 64, th * 512 : (th + 1) * 512],
                                    start=True,
                                    stop=True,
                                )
                            stats = sm.tile([128, 2, 6], F32, tag="stats", bufs=4)
                            nc.vector.bn_stats(out=stats[:, 0, :], in_=e_ps[:, 0:512])
                            nc.vector.bn_stats(out=stats[:, 1, :], in_=e_ps[:, 512:1024])
                            nc.vector.bn_aggr(out=mv2[:, i, :], in_=stats)
                        # sd = sqrt(var*S/(S-1) + D*eps) batched over the pair
                        c2 = sm.tile([128, 2], F32, tag="c", bufs=4)
                        nc.scalar.activation(
                            out=c2, in_=mv2[:, :, 1], func=AF.Sqrt, bias=eps_a, scale=SCORR
                        )
                        nc.vector.reciprocal(out=c2, in_=c2)
                        nc.vector.tensor_scalar_mul(c2, c2, ga_bc[h])
                        for i, st, e_ps in e_pair:
                            st4 = pair * 2 + i
                            p_sb = work.tile([128, S], F32, tag="big4k", bufs=14)
                            nc.scalar.activation(
                                out=p_sb, in_=e_ps, func=AF.Exp, bias=0.0,
                                scale=c2[:, i : i + 1],
                            )
                            for half in range(2):
                                pt_ps = ps.tile([128, 4, 128], F32, tag="pt", bufs=2)
                                for k in range(4):
                                    tj = half * 4 + k
                                    nc.tensor.transpose(
                                        pt_ps[:, k, :],
                                        p_sb[:, tj * 128 : (tj + 1) * 128],
                                        ident,
                                    )
                                nc.vector.tensor_copy(
                                    out=pT[:, half * 4 : half * 4 + 4,
                                           st4 * 128 : (st4 + 1) * 128],
                                    in_=pt_ps,
                                )
                    # AV for this half: [65, 512]; row 64 = softmax denominator
                    av_ps = ps.tile([DHD + 1, 512], F32, tag="acc", bufs=2)
                    for tj in range(8):
                        nc.tensor.matmul(
                            av_ps,
                            lhsT=v_sb[tj][:, h, :],
                            rhs=pT[:, tj, :],
                            start=(tj == 0),
                            stop=(tj == 7),
                        )
                    rrow = sm.tile([1, 512], F32, tag="rrow", bufs=4)
                    nc.vector.tensor_copy(out=rrow, in_=av_ps[DHD : DHD + 1, :])
                    nc.vector.reciprocal(out=rrow, in_=rrow)
                    rbc_ps = ps.tile([128, 512], F32, tag="pt", bufs=2)
                    nc.tensor.matmul(rbc_ps, lhsT=ones1, rhs=rrow, start=True, stop=True)
                    rbc_sb = sm.tile([128, 512], F32, tag="rbc", bufs=4)
                    nc.vector.tensor_copy(out=rbc_sb, in_=rbc_ps)
                    nc.vector.tensor_tensor(
                        out=aT[h // 2][hp : hp + 64, sh * 512 : (sh + 1) * 512],
                        in0=av_ps[0:DHD, :],
                        in1=rbc_sb[0:DHD, :],
                        op=mybir.AluOpType.mult,
                    )

            # ---------------- phase 3: WO + residual + LN1 ----------------
            x2_sb = [work.tile([128, D], F32, tag="v2k", bufs=9, name=f"x2_{i}") for i in range(N_ST)]
            x2T = [work.tile([128, S], F32, tag="big4k", bufs=14, name=f"x2T{i}") for i in range(3)]
            LCORR = float(D) / float(D - 1)

            def layer_norm(dst, src_ps, res_tiles, g_bc, b_bc, extra=None):
                # dst = LN(src_ps + residuals) * g + b   (src_ps in PSUM)
                xr = sm.tile([128, D], F32, tag="xr", bufs=4)
                nc.vector.tensor_add(xr, src_ps, res_tiles[0])
                for rt in res_tiles[1:]:
                    nc.vector.tensor_add(xr, xr, rt)
                stats = sm.tile([128, 6], F32, tag="lstats", bufs=4)
                nc.vector.bn_stats(out=stats, in_=xr)
                mv = sm.tile([128, 2], F32, tag="lmv", bufs=4)
                nc.vector.bn_aggr(out=mv, in_=stats)
                sd = sm.tile([128, 1], F32, tag="lsd", bufs=4)
                nc.scalar.activation(
                    out=sd, in_=mv[:, 1:2], func=AF.Sqrt, bias=eps_l, scale=LCORR
                )
                rstd = sm.tile([128, 1], F32, tag="lrstd", bufs=4)
                nc.vector.reciprocal(out=rstd, in_=sd)
                grstd = sm.tile([128, 1], F32, tag="lgr", bufs=4)
                nc.vector.tensor_mul(grstd, rstd, g_bc)
                nc.vector.tensor_scalar(
                    out=dst,
                    in0=xr,
                    scalar1=mv[:, 0:1],
                    scalar2=grstd,
                    op0=mybir.AluOpType.subtract,
                    op1=mybir.AluOpType.mult,
                )
                nc.vector.tensor_scalar_add(dst, dst, b_bc)

            for st in range(N_ST):
                x1_ps = ps.tile([128, D], F32, tag="acc", bufs=2)
                for it in range(4):
                    nc.tensor.matmul(
                        x1_ps,
                        lhsT=aT[it][:, st * 128 : (st + 1) * 128],
                        rhs=wo_sb[it],
                        start=(it == 0),
                        stop=(it == 3),
                    )
                layer_norm(x2_sb[st], x1_ps, [x_sb[:, st, :]], g1_bc, b1_bc)
                # transpose x2[st] -> x2T chunks
                xt_ps = ps.tile([128, 4, 128], F32, tag="pt", bufs=2)
                for jc, (j0, jn) in enumerate(J_CHUNKS):
                    nc.tensor.transpose(
                        xt_ps[:jn, jc, :], x2_sb[st][:, j0 : j0 + jn], ident
                    )
                for jc, (j0, jn) in enumerate(J_CHUNKS):
                    nc.vector.tensor_copy(
                        out=x2T[jc][:jn, st * 128 : (st + 1) * 128],
                        in_=xt_ps[:jn, jc, :],
                    )

            # ---------------- phase 4: FFN + LN2 ----------------
            h1T = [work.tile([128, S], F32, tag="big4k", bufs=14, name=f"h1T{i}") for i in range(10)]
            for mt, (m0, msz) in enumerate(M_CHUNKS):
                for sh in range(N_SH):
                    h1_ps = ps.tile([128, 512], F32, tag="acc", bufs=2)
                    for jc, (j0, jn) in enumerate(J_CHUNKS):
                        nc.tensor.matmul(
                            h1_ps[:msz, :],
                            lhsT=w1_sb[jc][:jn, m0 : m0 + msz],
                            rhs=x2T[jc][:jn, sh * 512 : (sh + 1) * 512],
                            start=(jc == 0),
                            stop=(jc == 2),
                        )
                    nc.scalar.activation(
                        out=h1T[mt][:msz, sh * 512 : (sh + 1) * 512],
                        in_=h1_ps[:msz, :],
                        func=AF.Relu,
                        bias=fb1_sb[mt][:msz, :],
                        scale=1.0,
                    )
            if dbg:
                for i in range(N_ST):
                    nc.sync.dma_start(out=dx2[i * 128 : (i + 1) * 128, :], in_=x2_sb[i])
                for mt, (m0, msz) in enumerate(M_CHUNKS):
                    nc.sync.dma_start(out=dh1[m0 : m0 + msz, :], in_=h1T[mt][:msz, :])
            for st in range(N_ST):
                h2_ps = ps.tile([128, D], F32, tag="acc", bufs=2)
                for mt, (m0, msz) in enumerate(M_CHUNKS):
                    nc.tensor.matmul(
                        h2_ps,
                        lhsT=h1T[mt][:msz, st * 128 : (st + 1) * 128],
                        rhs=w2_sb[mt][:msz, :],
                        start=(mt == 0),
                        stop=(mt == 9),
                    )
                o_sb = sm.tile([128, D], F32, tag="o", bufs=4)
                layer_norm(o_sb, h2_ps, [fb2_bc, x2_sb[st]], g2_bc, b2_bc)
                nc.sync.dma_start(out=outd[st * 128 : (st + 1) * 128, :], in_=o_sb)

    nc.compile()
    return nc


def _get_nc():
    if "nc" not in _cache:
        _cache["nc"] = _build_nc()
    return _cache["nc"]


def kernel(x, WQ, WK, WV, WO, W1, b1, W2, b2, gamma_a, beta_a,
           gamma1, beta1, gamma2, beta2):
    global _last_results
    f = np.float32
    x = np.asarray(x, f)

    def perm(W):
        # head h -> contiguous rows [h*64, (h+1)*64)
        return np.asarray(W, f).reshape(DHD, H, D).transpose(1, 0, 2).reshape(DH, D)

    wq_t = np.ascontiguousarray(perm(WQ).T)
    wk_t = np.ascontiguousarray(perm(WK).T)
    wv_t = np.ascontiguousarray(perm(WV).T)
    wo = np.ascontiguousarray(np.asarray(WO, f))
    w1 = np.ascontiguousarray(np.asarray(W1, f))
    w2 = np.ascontiguousarray(np.asarray(W2, f))
    fb1 = np.zeros((1280, 1), f)
    fb1[:DF, 0] = np.asarray(b1, f)
    fb2 = np.ascontiguousarray(np.asarray(b2, f))
    # beta_a drops out of softmax (per-row constant shift); the 1/sqrt(D)
    # score scale cancels inside the score LayerNorm: softmax(g*LN(e/sqrt(D)))
    # == softmax(g/sqrt(var(e) + D*eps) * e), so gamma is used unscaled and
    # D*eps replaces eps on-device.
    ga = np.ascontiguousarray(np.asarray(gamma_a, f).reshape(H, 1))
    ln = np.array(
        [np.asarray(gamma1, f), np.asarray(beta1, f),
         np.asarray(gamma2, f), np.asarray(beta2, f)], f
    ).reshape(4, 1)

    shared = {"wq": wq_t, "wk": wk_t, "wv": wv_t, "wo": wo, "w1": w1, "w2": w2,
              "fb1": fb1, "fb2": fb2, "ga": ga, "ln": ln}
    in_maps = []
    for b in range(B):
        xb = np.ascontiguousarray(x[b])
        in_maps.append({"x": xb, "xt": np.ascontiguousarray(xb.T), **shared})

    nc = _get_nc()
    res = run_bass_kernel_spmd(nc, in_maps, core_ids=list(range(NCORES)), trace=TRACE)
    _last_results = res
    return np.stack([res.results[b]["out"] for b in range(B)], axis=0)


# revision 9
# speedup vs baseline: 1.7679x; 1.0748x over previous
"""Trainium2 Bass kernel: 8-head transformer encoder layer (B=8, S=1024,
D=300, Dh=512, H=8), data-parallel over batch across 8 NeuronCores.

Per core (one batch element):
  qT/kT = Wp @ x^T  (heads contiguous via host-side weight-row permute)
  v     = x @ Wp^T
  per head: e = q k^T (PSUM) -> bn_stats var -> c = gamma*sqrt(D)/sd
            p = exp(c*e) (ACT, accum row-sum r) -> p *= 1/r (GPSIMD)
            pT via PE transpose -> heads^T = v^T-chunks @ pT (PSUM acc)
  x1 = a @ WO ; x2 = LN(x1 + x) ; x2T via PE transpose
  h1T = relu(W1^T-form @ x2T + b1) ; h2 = h1T-chunks @ W2
  out = LN(h2 + b2 + x2)
"""

import math

import numpy as np

import concourse.bass as bass
import concourse.tile as tile
from concourse import bacc, mybir
from concourse.bass_utils import run_bass_kernel_spmd
from concourse.masks import make_identity

F32 = mybir.dt.float32
AF = mybir.ActivationFunctionType

B, S, D, DH, H, DHD = 8, 1024, 300, 512, 8, 64
DF = 4 * D  # 1200
EPS = 1e-8
NCORES = 8

J_CHUNKS = [(0, 128), (128, 128), (256, 44)]  # D=300 partition chunks
M_CHUNKS = [(i * 128, min(128, DF - i * 128)) for i in range(10)]  # DF=1200
N_ST = S // 128  # 8 s-tiles
N_SH = S // 512  # 2 s-halves

TRACE = False
_cache = {}
_last_results = None


def _build_nc(dbg=False):
    nc = bacc.Bacc("TRN2", debug=False)

    xd = nc.dram_tensor("x", [S, D], F32, kind="ExternalInput").ap()
    xtd = nc.dram_tensor("xt", [D, S], F32, kind="ExternalInput").ap()
    wqd = nc.dram_tensor("wq", [D, DH], F32, kind="ExternalInput").ap()
    wkd = nc.dram_tensor("wk", [D, DH], F32, kind="ExternalInput").ap()
    wvd = nc.dram_tensor("wv", [D, DH], F32, kind="ExternalInput").ap()
    wod = nc.dram_tensor("wo", [DH, D], F32, kind="ExternalInput").ap()
    w1d = nc.dram_tensor("w1", [D, DF], F32, kind="ExternalInput").ap()
    w2d = nc.dram_tensor("w2", [DF, D], F32, kind="ExternalInput").ap()
    fb1d = nc.dram_tensor("fb1", [1280, 1], F32, kind="ExternalInput").ap()
    fb2d = nc.dram_tensor("fb2", [D], F32, kind="ExternalInput").ap()
    gad = nc.dram_tensor("ga", [H, 1], F32, kind="ExternalInput").ap()
    lnd = nc.dram_tensor("ln", [4, 1], F32, kind="ExternalInput").ap()
    outd = nc.dram_tensor("out", [S, D], F32, kind="ExternalOutput").ap()
    if dbg:
        dqT = nc.dram_tensor("dqT", [DH, S], F32, kind="ExternalOutput").ap()
        dkT = nc.dram_tensor("dkT", [DH, S], F32, kind="ExternalOutput").ap()
        dv = nc.dram_tensor("dv", [S, DH], F32, kind="ExternalOutput").ap()
        dp0 = nc.dram_tensor("dp0", [S, S], F32, kind="ExternalOutput").ap()
        daT = nc.dram_tensor("daT", [DH, S], F32, kind="ExternalOutput").ap()
        dx2 = nc.dram_tensor("dx2", [S, D], F32, kind="ExternalOutput").ap()
        dh1 = nc.dram_tensor("dh1", [DF, S], F32, kind="ExternalOutput").ap()
        dr = nc.dram_tensor("dr", [H, S], F32, kind="ExternalOutput").ap()
        dc = nc.dram_tensor("dc", [H, S], F32, kind="ExternalOutput").ap()
        dmv = nc.dram_tensor("dmv", [H, S, 2], F32, kind="ExternalOutput").ap()

    with tile.TileContext(nc) as tc:
        with (
            tc.tile_pool(name="wts", bufs=1) as wts,
            tc.tile_pool(name="work", bufs=1) as work,
            tc.tile_pool(name="sm", bufs=8) as sm,
            tc.tile_pool(name="ps", bufs=1, space="PSUM") as ps,
        ):
            # ---------------- constant / weight loads ----------------
            ident = wts.tile([128, 128], F32, tag="ident")
            make_identity(nc, ident)

            ones1 = wts.tile([1, 128], F32, tag="ones1")
            nc.vector.memset(ones1, 1.0)

            eps_a = wts.tile([128, 1], F32, tag="eps_a")  # D*EPS (score LN)
            nc.vector.memset(eps_a, D * EPS)
            eps_l = wts.tile([128, 1], F32, tag="eps_l")  # EPS (x LNs)
            nc.vector.memset(eps_l, EPS)

            def bcast_load(src_ap, shape, tag):
                t = wts.tile(shape, F32, tag=tag)
                nc.sync.dma_start(out=t, in_=src_ap.to_broadcast(shape))
                return t

            ga_bc = [bcast_load(gad[h : h + 1, :], [128, 1], f"ga{h}") for h in range(H)]
            g1_bc = bcast_load(lnd[0:1, :], [128, 1], "g1")
            b1_bc = bcast_load(lnd[1:2, :], [128, 1], "b1")
            g2_bc = bcast_load(lnd[2:3, :], [128, 1], "g2")
            b2_bc = bcast_load(lnd[3:4, :], [128, 1], "b2")
            fb2_bc = wts.tile([128, D], F32, tag="fb2")
            nc.sync.dma_start(
                out=fb2_bc,
                in_=bass.AP(tensor=fb2d.tensor, offset=fb2d.offset, ap=[[0, 128]] + list(fb2d.ap)),
            )
            fb1_sb = []
            for mt, (m0, msz) in enumerate(M_CHUNKS):
                t = wts.tile([128, 1], F32, tag=f"fb1_{mt}")
                nc.sync.dma_start(out=t[:msz, :], in_=fb1d[m0 : m0 + msz, :])
                fb1_sb.append(t)

            # x natural: [128, 8, 300] (partition = s % 128)
            x_sb = wts.tile([128, N_ST, D], F32, tag="x")
            nc.sync.dma_start(out=x_sb, in_=xd.rearrange("(n p) d -> p n d", p=128))

            def chunked_load(src, width, tag):
                tiles = []
                for jc, (j0, jn) in enumerate(J_CHUNKS):
                    t = wts.tile([128, width], F32, tag=f"{tag}{jc}")
                    nc.sync.dma_start(out=t[:jn, :], in_=src[j0 : j0 + jn, :])
                    tiles.append(t)
                return tiles

            xt_sb = chunked_load(xtd, S, "xt")    # [300, 1024] in 3 chunks
            wq_sb = chunked_load(wqd, DH, "wq")   # [300, 512]
            wk_sb = chunked_load(wkd, DH, "wk")
            wv_sb = chunked_load(wvd, DH, "wv")
            w1_sb = chunked_load(w1d, DF, "w1")   # [300, 1200]

            wo_sb = []
            for it in range(4):
                t = wts.tile([128, D], F32, tag=f"wo{it}")
                nc.sync.dma_start(out=t, in_=wod[it * 128 : (it + 1) * 128, :])
                wo_sb.append(t)
            w2_sb = []
            for mt, (m0, msz) in enumerate(M_CHUNKS):
                t = wts.tile([128, D], F32, tag=f"w2_{mt}")
                nc.sync.dma_start(out=t[:msz, :], in_=w2d[m0 : m0 + msz, :])
                w2_sb.append(t)

            # ---------------- phase 1: projections ----------------
            # qT/kT [Dh, S] as 4 x [128, 1024];  v [S, Dh] as 8 x [128, 512]
            qT = [work.tile([128, S], F32, tag="big4k", bufs=14, name=f"qT{i}") for i in range(4)]
            kT = [work.tile([128, S], F32, tag="big4k", bufs=14, name=f"kT{i}") for i in range(4)]
            v_sb = [work.tile([128, H, DHD + 1], F32, tag="v2k", bufs=9, name=f"v{i}") for i in range(N_ST)]

            for dst, w in ((qT, wq_sb), (kT, wk_sb)):
                for dt in range(4):
                    for sh in range(N_SH):
                        pp = ps.tile([128, 512], F32, tag="e", bufs=6)
                        for jc, (j0, jn) in enumerate(J_CHUNKS):
                            nc.tensor.matmul(
                                pp,
                                lhsT=w[jc][:jn, dt * 128 : (dt + 1) * 128],
                                rhs=xt_sb[jc][:jn, sh * 512 : (sh + 1) * 512],
                                start=(jc == 0),
                                stop=(jc == 2),
                            )
                        nc.vector.tensor_copy(out=dst[dt][:, sh * 512 : (sh + 1) * 512], in_=pp)
            for st in range(N_ST):
                pp = ps.tile([128, 512], F32, tag="e", bufs=6)
                for jc, (j0, jn) in enumerate(J_CHUNKS):
                    nc.tensor.matmul(
                        pp,
                        lhsT=xt_sb[jc][:jn, st * 128 : (st + 1) * 128],
                        rhs=wv_sb[jc][:jn, :],
                        start=(jc == 0),
                        stop=(jc == 2),
                    )
                nc.vector.tensor_copy(
                    out=v_sb[st][:, :, 0:DHD],
                    in_=pp.rearrange("p (h d) -> p h d", h=H),
                )
                nc.vector.memset(v_sb[st][:, :, DHD : DHD + 1], 1.0)

            if dbg:
                for i in range(4):
                    nc.sync.dma_start(out=dqT[i * 128 : (i + 1) * 128, :], in_=qT[i])
                    nc.sync.dma_start(out=dkT[i * 128 : (i + 1) * 128, :], in_=kT[i])
                for i in range(N_ST):
                    nc.sync.dma_start(out=dv[i * 128 : (i + 1) * 128, :], in_=v_sb[i][:, :, 0:DHD])

            # ---------------- phase 2: attention ----------------
            aT = [work.tile([128, S], F32, tag="big4k", bufs=14, name=f"aT{i}") for i in range(4)]
            SCORR = float(S) / float(S - 1)

            for h in range(H):
                qt_t, hp = qT[h // 2], (h % 2) * 64
                kt_t = kT[h // 2]
                for sh in range(N_SH):
                    pT = work.tile([128, 8, 512], F32, tag="pt16k", bufs=2)
                    for pair in range(2):
                        e_pair = []
                        mv2 = sm.tile([128, 2, 2], F32, tag="mv", bufs=4)
                        for i in range(2):
                            st = sh * 4 + pair * 2 + i
                            eh0 = ps.tile([128, 512], F32, tag="e", bufs=6, name="eh0")
                            eh1 = ps.tile([128, 512], F32, tag="e", bufs=6, name="eh1")
                            e_pair.append((i, st, eh0, eh1))
                            stats = sm.tile([128, 2, 6], F32, tag="stats", bufs=4)
                            for th, eh in ((0, eh0), (1, eh1)):
                                nc.tensor.matmul(
                                    eh,
                                    lhsT=qt_t[hp : hp + 64, st * 128 : (st + 1) * 128],
                                    rhs=kt_t[hp : hp + 64, th * 512 : (th + 1) * 512],
                                    start=True,
                                    stop=True,
                                )
                                nc.vector.bn_stats(out=stats[:, th, :], in_=eh)
                            nc.vector.bn_aggr(out=mv2[:, i, :], in_=stats)
                        # sd = sqrt(var*S/(S-1) + D*eps) batched over the pair
                        c2 = sm.tile([128, 2], F32, tag="c", bufs=4)
                        nc.scalar.activation(
                            out=c2, in_=mv2[:, :, 1], func=AF.Sqrt, bias=eps_a, scale=SCORR
                        )
                        nc.vector.reciprocal(out=c2, in_=c2)
                        nc.vector.tensor_scalar_mul(c2, c2, ga_bc[h])
                        for i, st, eh0, eh1 in e_pair:
                            st4 = pair * 2 + i
                            p_sb = work.tile([128, S], F32, tag="big4k", bufs=14)
                            for th, eh in ((0, eh0), (1, eh1)):
                                nc.scalar.activation(
                                    out=p_sb[:, th * 512 : (th + 1) * 512],
                                    in_=eh, func=AF.Exp, bias=0.0,
                                    scale=c2[:, i : i + 1],
                                )
                            for half in range(2):
                                pt_ps = ps.tile([128, 4, 128], F32, tag="pt", bufs=2)
                                for k in range(4):
                                    tj = half * 4 + k
                                    nc.tensor.transpose(
                                        pt_ps[:, k, :],
                                        p_sb[:, tj * 128 : (tj + 1) * 128],
                                        ident,
                                    )
                                nc.vector.tensor_copy(
                                    out=pT[:, half * 4 : half * 4 + 4,
                                           st4 * 128 : (st4 + 1) * 128],
                                    in_=pt_ps,
                                )
                    # AV for this half: [65, 512]; row 64 = softmax denominator
                    av_ps = ps.tile([DHD + 1, 512], F32, tag="e", bufs=6)
                    for tj in range(8):
                        nc.tensor.matmul(
                            av_ps,
                            lhsT=v_sb[tj][:, h, :],
                            rhs=pT[:, tj, :],
                            start=(tj == 0),
                            stop=(tj == 7),
                        )
                    rrow = sm.tile([1, 512], F32, tag="rrow", bufs=4)
                    nc.vector.tensor_copy(out=rrow, in_=av_ps[DHD : DHD + 1, :])
                    nc.vector.reciprocal(out=rrow, in_=rrow)
                    rbc_ps = ps.tile([128, 512], F32, tag="pt", bufs=2)
                    nc.tensor.matmul(rbc_ps, lhsT=ones1, rhs=rrow, start=True, stop=True)
                    rbc_sb = sm.tile([128, 512], F32, tag="rbc", bufs=4)
                    nc.vector.tensor_copy(out=rbc_sb, in_=rbc_ps)
                    nc.vector.tensor_tensor(
                        out=aT[h // 2][hp : hp + 64, sh * 512 : (sh + 1) * 512],
                        in0=av_ps[0:DHD, :],
                        in1=rbc_sb[0:DHD, :],
                        op=mybir.AluOpType.mult,
                    )

            # ---------------- phase 3: WO + residual + LN1 ----------------
            x2_sb = [work.tile([128, D], F32, tag="v2k", bufs=9, name=f"x2_{i}") for i in range(N_ST)]
            x2T = [work.tile([128, S], F32, tag="big4k", bufs=14, name=f"x2T{i}") for i in range(3)]
            LCORR = float(D) / float(D - 1)

            def layer_norm(dst, src_ps, res_tiles, g_bc, b_bc, extra=None):
                # dst = LN(src_ps + residuals) * g + b   (src_ps in PSUM)
                xr = sm.tile([128, D], F32, tag="xr", bufs=4)
                nc.vector.tensor_add(xr, src_ps, res_tiles[0])
                for rt in res_tiles[1:]:
                    nc.vector.tensor_add(xr, xr, rt)
                stats = sm.tile([128, 6], F32, tag="lstats", bufs=4)
                nc.vector.bn_stats(out=stats, in_=xr)
                mv = sm.tile([128, 2], F32, tag="lmv", bufs=4)
                nc.vector.bn_aggr(out=mv, in_=stats)
                sd = sm.tile([128, 1], F32, tag="lsd", bufs=4)
                nc.scalar.activation(
                    out=sd, in_=mv[:, 1:2], func=AF.Sqrt, bias=eps_l, scale=LCORR
                )
                rstd = sm.tile([128, 1], F32, tag="lrstd", bufs=4)
                nc.vector.reciprocal(out=rstd, in_=sd)
                grstd = sm.tile([128, 1], F32, tag="lgr", bufs=4)
                nc.vector.tensor_mul(grstd, rstd, g_bc)
                nc.vector.tensor_scalar(
                    out=dst,
                    in0=xr,
                    scalar1=mv[:, 0:1],
                    scalar2=grstd,
                    op0=mybir.AluOpType.subtract,
                    op1=mybir.AluOpType.mult,
                )
                nc.vector.tensor_scalar_add(dst, dst, b_bc)

            for st in range(N_ST):
                x1_ps = ps.tile([128, D], F32, tag="e", bufs=6)
                for it in range(4):
                    nc.tensor.matmul(
                        x1_ps,
                        lhsT=aT[it][:, st * 128 : (st + 1) * 128],
                        rhs=wo_sb[it],
                        start=(it == 0),
                        stop=(it == 3),
                    )
                layer_norm(x2_sb[st], x1_ps, [x_sb[:, st, :]], g1_bc, b1_bc)
                # transpose x2[st] -> x2T chunks
                xt_ps = ps.tile([128, 4, 128], F32, tag="pt", bufs=2)
                for jc, (j0, jn) in enumerate(J_CHUNKS):
                    nc.tensor.transpose(
                        xt_ps[:jn, jc, :], x2_sb[st][:, j0 : j0 + jn], ident
                    )
                for jc, (j0, jn) in enumerate(J_CHUNKS):
                    nc.vector.tensor_copy(
                        out=x2T[jc][:jn, st * 128 : (st + 1) * 128],
                        in_=xt_ps[:jn, jc, :],
                    )

            # ---------------- phase 4: FFN + LN2 ----------------
            h1T = [work.tile([128, S], F32, tag="big4k", bufs=14, name=f"h1T{i}") for i in range(10)]
            for mt, (m0, msz) in enumerate(M_CHUNKS):
                for sh in range(N_SH):
                    h1_ps = ps.tile([128, 512], F32, tag="e", bufs=6)
                    for jc, (j0, jn) in enumerate(J_CHUNKS):
                        nc.tensor.matmul(
                            h1_ps[:msz, :],
                            lhsT=w1_sb[jc][:jn, m0 : m0 + msz],
                            rhs=x2T[jc][:jn, sh * 512 : (sh + 1) * 512],
                            start=(jc == 0),
                            stop=(jc == 2),
                        )
                    nc.scalar.activation(
                        out=h1T[mt][:msz, sh * 512 : (sh + 1) * 512],
                        in_=h1_ps[:msz, :],
                        func=AF.Relu,
                        bias=fb1_sb[mt][:msz, :],
                        scale=1.0,
                    )
            if dbg:
                for i in range(N_ST):
                    nc.sync.dma_start(out=dx2[i * 128 : (i + 1) * 128, :], in_=x2_sb[i])
                for mt, (m0, msz) in enumerate(M_CHUNKS):
                    nc.sync.dma_start(out=dh1[m0 : m0 + msz, :], in_=h1T[mt][:msz, :])
            for st in range(N_ST):
                h2_ps = ps.tile([128, D], F32, tag="e", bufs=6)
                for mt, (m0, msz) in enumerate(M_CHUNKS):
                    nc.tensor.matmul(
                        h2_ps,
                        lhsT=h1T[mt][:msz, st * 128 : (st + 1) * 128],
                        rhs=w2_sb[mt][:msz, :],
                        start=(mt == 0),
                        stop=(mt == 9),
                    )
                o_sb = sm.tile([128, D], F32, tag="o", bufs=4)
                layer_norm(o_sb, h2_ps, [fb2_bc, x2_sb[st]], g2_bc, b2_bc)
                nc.sync.dma_start(out=outd[st * 128 : (st + 1) * 128, :], in_=o_sb)

    nc.compile()
    return nc


def _get_nc():
    if "nc" not in _cache:
        _cache["nc"] = _build_nc()
    return _cache["nc"]


def kernel(x, WQ, WK, WV, WO, W1, b1, W2, b2, gamma_a, beta_a,
           gamma1, beta1, gamma2, beta2):
    global _last_results
    f = np.float32
    x = np.asarray(x, f)

    def perm(W):
        # head h -> contiguous rows [h*64, (h+1)*64)
        return np.asarray(W, f).reshape(DHD, H, D).transpose(1, 0, 2).reshape(DH, D)

    wq_t = np.ascontiguousarray(perm(WQ).T)
    wk_t = np.ascontiguousarray(perm(WK).T)
    wv_t = np.ascontiguousarray(perm(WV).T)
    wo = np.ascontiguousarray(np.asarray(WO, f))
    w1 = np.ascontiguousarray(np.asarray(W1, f))
    w2 = np.ascontiguousarray(np.asarray(W2, f))
    fb1 = np.zeros((1280, 1), f)
    fb1[:DF, 0] = np.asarray(b1, f)
    fb2 = np.ascontiguousarray(np.asarray(b2, f))
    # beta_a drops out of softmax (per-row constant shift); the 1/sqrt(D)
    # score scale cancels inside the score LayerNorm: softmax(g*LN(e/sqrt(D)))
    # == softmax(g/sqrt(var(e) + D*eps) * e), so gamma is used unscaled and
    # D*eps replaces eps on-device.
    ga = np.ascontiguousarray(np.asarray(gamma_a, f).reshape(H, 1))
    ln = np.array(
        [np.asarray(gamma1, f), np.asarray(beta1, f),
         np.asarray(gamma2, f), np.asarray(beta2, f)], f
    ).reshape(4, 1)

    shared = {"wq": wq_t, "wk": wk_t, "wv": wv_t, "wo": wo, "w1": w1, "w2": w2,
              "fb1": fb1, "fb2": fb2, "ga": ga, "ln": ln}
    in_maps = []
    for b in range(B):
        xb = np.ascontiguousarray(x[b])
        in_maps.append({"x": xb, "xt": np.ascontiguousarray(xb.T), **shared})

    nc = _get_nc()
    res = run_bass_kernel_spmd(nc, in_maps, core_ids=list(range(NCORES)), trace=TRACE)
    _last_results = res
    return np.stack([res.results[b]["out"] for b in range(B)], axis=0)


# revision 10
# speedup vs baseline: 1.8493x; 1.0460x over previous
"""Trainium2 Bass kernel: 8-head transformer encoder layer (B=8, S=1024,
D=300, Dh=512, H=8), data-parallel over batch across 8 NeuronCores.

Per core (one batch element):
  qT/kT = Wp @ x^T  (heads contiguous via host-side weight-row permute)
  v     = x @ Wp^T
  per head: e = q k^T (PSUM) -> bn_stats var -> c = gamma*sqrt(D)/sd
            p = exp(c*e) (ACT, accum row-sum r) -> p *= 1/r (GPSIMD)
            pT via PE transpose -> heads^T = v^T-chunks @ pT (PSUM acc)
  x1 = a @ WO ; x2 = LN(x1 + x) ; x2T via PE transpose
  h1T = relu(W1^T-form @ x2T + b1) ; h2 = h1T-chunks @ W2
  out = LN(h2 + b2 + x2)
"""

import math

import numpy as np

import concourse.bass as bass
import concourse.tile as tile
from concourse import bacc, mybir
from concourse.bass_utils import run_bass_kernel_spmd
from concourse.masks import make_identity

F32 = mybir.dt.float32
AF = mybir.ActivationFunctionType

B, S, D, DH, H, DHD = 8, 1024, 300, 512, 8, 64
DF = 4 * D  # 1200
EPS = 1e-8
NCORES = 8

J_CHUNKS = [(0, 128), (128, 128), (256, 44)]  # D=300 partition chunks
M_CHUNKS = [(i * 128, min(128, DF - i * 128)) for i in range(10)]  # DF=1200
N_ST = S // 128  # 8 s-tiles
N_SH = S // 512  # 2 s-halves

TRACE = False
_cache = {}
_last_results = None


def _build_nc(dbg=False):
    nc = bacc.Bacc("TRN2", debug=False)

    xd = nc.dram_tensor("x", [S, D], F32, kind="ExternalInput").ap()
    xtd = nc.dram_tensor("xt", [D, S], F32, kind="ExternalInput").ap()
    wqd = nc.dram_tensor("wq", [D, DH], F32, kind="ExternalInput").ap()
    wkd = nc.dram_tensor("wk", [D, DH], F32, kind="ExternalInput").ap()
    wvd = nc.dram_tensor("wv", [D, DH], F32, kind="ExternalInput").ap()
    wod = nc.dram_tensor("wo", [DH, D], F32, kind="ExternalInput").ap()
    w1d = nc.dram_tensor("w1", [D, DF], F32, kind="ExternalInput").ap()
    w2d = nc.dram_tensor("w2", [DF, D], F32, kind="ExternalInput").ap()
    fb1d = nc.dram_tensor("fb1", [1280, 1], F32, kind="ExternalInput").ap()
    fb2d = nc.dram_tensor("fb2", [D], F32, kind="ExternalInput").ap()
    gad = nc.dram_tensor("ga", [H, 1], F32, kind="ExternalInput").ap()
    lnd = nc.dram_tensor("ln", [4, 1], F32, kind="ExternalInput").ap()
    outd = nc.dram_tensor("out", [S, D], F32, kind="ExternalOutput").ap()
    if dbg:
        dqT = nc.dram_tensor("dqT", [DH, S], F32, kind="ExternalOutput").ap()
        dkT = nc.dram_tensor("dkT", [DH, S], F32, kind="ExternalOutput").ap()
        dv = nc.dram_tensor("dv", [S, DH], F32, kind="ExternalOutput").ap()
        dp0 = nc.dram_tensor("dp0", [S, S], F32, kind="ExternalOutput").ap()
        daT = nc.dram_tensor("daT", [DH, S], F32, kind="ExternalOutput").ap()
        dx2 = nc.dram_tensor("dx2", [S, D], F32, kind="ExternalOutput").ap()
        dh1 = nc.dram_tensor("dh1", [DF, S], F32, kind="ExternalOutput").ap()
        dr = nc.dram_tensor("dr", [H, S], F32, kind="ExternalOutput").ap()
        dc = nc.dram_tensor("dc", [H, S], F32, kind="ExternalOutput").ap()
        dmv = nc.dram_tensor("dmv", [H, S, 2], F32, kind="ExternalOutput").ap()

    with tile.TileContext(nc) as tc:
        with (
            tc.tile_pool(name="wts", bufs=1) as wts,
            tc.tile_pool(name="work", bufs=1) as work,
            tc.tile_pool(name="sm", bufs=8) as sm,
            tc.tile_pool(name="ps", bufs=1, space="PSUM") as ps,
        ):
            # ---------------- constant / weight loads ----------------
            ident = wts.tile([128, 128], F32, tag="ident")
            make_identity(nc, ident)

            ones1 = wts.tile([1, 128], F32, tag="ones1")
            nc.vector.memset(ones1, 1.0)
            dummy = wts.tile([128, 1], F32, tag="dummy")
            nc.vector.memset(dummy, 1.0)
            dsink = wts.tile([128, 1], F32, tag="dsink")

            eps_a = wts.tile([128, 1], F32, tag="eps_a")  # D*EPS (score LN)
            nc.vector.memset(eps_a, D * EPS)
            eps_l = wts.tile([128, 1], F32, tag="eps_l")  # EPS (x LNs)
            nc.vector.memset(eps_l, EPS)

            def bcast_load(src_ap, shape, tag):
                t = wts.tile(shape, F32, tag=tag)
                nc.sync.dma_start(out=t, in_=src_ap.to_broadcast(shape))
                return t

            ga_bc = [bcast_load(gad[h : h + 1, :], [128, 1], f"ga{h}") for h in range(H)]
            g1_bc = bcast_load(lnd[0:1, :], [128, 1], "g1")
            b1_bc = bcast_load(lnd[1:2, :], [128, 1], "b1")
            g2_bc = bcast_load(lnd[2:3, :], [128, 1], "g2")
            b2_bc = bcast_load(lnd[3:4, :], [128, 1], "b2")
            fb2_bc = wts.tile([128, D], F32, tag="fb2")
            nc.sync.dma_start(
                out=fb2_bc,
                in_=bass.AP(tensor=fb2d.tensor, offset=fb2d.offset, ap=[[0, 128]] + list(fb2d.ap)),
            )
            fb1_sb = []
            for mt, (m0, msz) in enumerate(M_CHUNKS):
                t = wts.tile([128, 1], F32, tag=f"fb1_{mt}")
                nc.sync.dma_start(out=t[:msz, :], in_=fb1d[m0 : m0 + msz, :])
                fb1_sb.append(t)

            # x natural: [128, 8, 300] (partition = s % 128)
            x_sb = wts.tile([128, N_ST, D], F32, tag="x")
            nc.sync.dma_start(out=x_sb, in_=xd.rearrange("(n p) d -> p n d", p=128))

            def chunked_load(src, width, tag):
                tiles = []
                for jc, (j0, jn) in enumerate(J_CHUNKS):
                    t = wts.tile([128, width], F32, tag=f"{tag}{jc}")
                    nc.sync.dma_start(out=t[:jn, :], in_=src[j0 : j0 + jn, :])
                    tiles.append(t)
                return tiles

            xt_sb = chunked_load(xtd, S, "xt")    # [300, 1024] in 3 chunks
            wq_sb = chunked_load(wqd, DH, "wq")   # [300, 512]
            wk_sb = chunked_load(wkd, DH, "wk")
            wv_sb = chunked_load(wvd, DH, "wv")
            w1_sb = chunked_load(w1d, DF, "w1")   # [300, 1200]

            wo_sb = []
            for it in range(4):
                t = wts.tile([128, D], F32, tag=f"wo{it}")
                nc.sync.dma_start(out=t, in_=wod[it * 128 : (it + 1) * 128, :])
                wo_sb.append(t)
            w2_sb = []
            for mt, (m0, msz) in enumerate(M_CHUNKS):
                t = wts.tile([128, D], F32, tag=f"w2_{mt}")
                nc.sync.dma_start(out=t[:msz, :], in_=w2d[m0 : m0 + msz, :])
                w2_sb.append(t)

            # ---------------- phase 1: projections ----------------
            # qT/kT [Dh, S] as 4 x [128, 1024];  v [S, Dh] as 8 x [128, 512]
            qT = [work.tile([128, S], F32, tag="big4k", bufs=14, name=f"qT{i}") for i in range(4)]
            kT = [work.tile([128, S], F32, tag="big4k", bufs=14, name=f"kT{i}") for i in range(4)]
            v_sb = [work.tile([128, H, DHD + 1], F32, tag="v2k", bufs=9, name=f"v{i}") for i in range(N_ST)]

            for dst, w in ((qT, wq_sb), (kT, wk_sb)):
                for dt in range(4):
                    for sh in range(N_SH):
                        pp = ps.tile([128, 512], F32, tag="e", bufs=6)
                        for jc, (j0, jn) in enumerate(J_CHUNKS):
                            nc.tensor.matmul(
                                pp,
                                lhsT=w[jc][:jn, dt * 128 : (dt + 1) * 128],
                                rhs=xt_sb[jc][:jn, sh * 512 : (sh + 1) * 512],
                                start=(jc == 0),
                                stop=(jc == 2),
                            )
                        nc.vector.tensor_copy(out=dst[dt][:, sh * 512 : (sh + 1) * 512], in_=pp)
            for st in range(N_ST):
                pp = ps.tile([128, 512], F32, tag="e", bufs=6)
                for jc, (j0, jn) in enumerate(J_CHUNKS):
                    nc.tensor.matmul(
                        pp,
                        lhsT=xt_sb[jc][:jn, st * 128 : (st + 1) * 128],
                        rhs=wv_sb[jc][:jn, :],
                        start=(jc == 0),
                        stop=(jc == 2),
                    )
                nc.vector.tensor_copy(
                    out=v_sb[st][:, :, 0:DHD],
                    in_=pp.rearrange("p (h d) -> p h d", h=H),
                )
                nc.vector.memset(v_sb[st][:, :, DHD : DHD + 1], 1.0)

            if dbg:
                for i in range(4):
                    nc.sync.dma_start(out=dqT[i * 128 : (i + 1) * 128, :], in_=qT[i])
                    nc.sync.dma_start(out=dkT[i * 128 : (i + 1) * 128, :], in_=kT[i])
                for i in range(N_ST):
                    nc.sync.dma_start(out=dv[i * 128 : (i + 1) * 128, :], in_=v_sb[i][:, :, 0:DHD])

            # ---------------- phase 2: attention ----------------
            aT = [work.tile([128, S], F32, tag="big4k", bufs=14, name=f"aT{i}") for i in range(4)]
            SCORR = float(S) / float(S - 1)

            for h in range(H):
                qt_t, hp = qT[h // 2], (h % 2) * 64
                kt_t = kT[h // 2]
                for sh in range(N_SH):
                    pT = work.tile([128, 8, 512], F32, tag="pt16k", bufs=2)
                    for pair in range(2):
                        e_pair = []
                        mv2 = sm.tile([128, 2, 2], F32, tag="mv", bufs=4)
                        for i in range(2):
                            st = sh * 4 + pair * 2 + i
                            eh0 = ps.tile([128, 512], F32, tag="e", bufs=6, name="eh0")
                            eh1 = ps.tile([128, 512], F32, tag="e", bufs=6, name="eh1")
                            e_pair.append((i, st, eh0, eh1))
                            stats = sm.tile([128, 2, 6], F32, tag="stats", bufs=4)
                            for th, eh in ((0, eh0), (1, eh1)):
                                nc.tensor.matmul(
                                    eh,
                                    lhsT=qt_t[hp : hp + 64, st * 128 : (st + 1) * 128],
                                    rhs=kt_t[hp : hp + 64, th * 512 : (th + 1) * 512],
                                    start=True,
                                    stop=True,
                                )
                                nc.vector.bn_stats(out=stats[:, th, :], in_=eh)
                            nc.vector.bn_aggr(out=mv2[:, i, :], in_=stats)
                        # sd = sqrt(var*S/(S-1) + D*eps) batched over the pair
                        c2 = sm.tile([128, 2], F32, tag="c", bufs=4)
                        nc.scalar.activation(
                            out=c2, in_=mv2[:, :, 1], func=AF.Sqrt, bias=eps_a, scale=SCORR
                        )
                        nc.scalar.activation(
                            out=dsink, in_=dummy, func=AF.Exp, bias=0.0, scale=1.0
                        )
                        nc.vector.reciprocal(out=c2, in_=c2)
                        nc.vector.tensor_scalar_mul(c2, c2, ga_bc[h])
                        for i, st, eh0, eh1 in e_pair:
                            st4 = pair * 2 + i
                            p_sb = work.tile([128, S], F32, tag="big4k", bufs=14)
                            for th, eh in ((0, eh0), (1, eh1)):
                                nc.scalar.activation(
                                    out=p_sb[:, th * 512 : (th + 1) * 512],
                                    in_=eh, func=AF.Exp, bias=0.0,
                                    scale=c2[:, i : i + 1],
                                )
                            if i == 1:
                                nc.scalar.activation(
                                    out=dsink, in_=dummy, func=AF.Sqrt,
                                    bias=0.0, scale=1.0,
                                )
                            for half in range(2):
                                pt_ps = ps.tile([128, 4, 128], F32, tag="pt", bufs=2)
                                for k in range(4):
                                    tj = half * 4 + k
                                    nc.tensor.transpose(
                                        pt_ps[:, k, :],
                                        p_sb[:, tj * 128 : (tj + 1) * 128],
                                        ident,
                                    )
                                nc.vector.tensor_copy(
                                    out=pT[:, half * 4 : half * 4 + 4,
                                           st4 * 128 : (st4 + 1) * 128],
                                    in_=pt_ps,
                                )
                    # AV for this half: [65, 512]; row 64 = softmax denominator
                    av_ps = ps.tile([DHD + 1, 512], F32, tag="pt", bufs=2)
                    for tj in range(8):
                        nc.tensor.matmul(
                            av_ps,
                            lhsT=v_sb[tj][:, h, :],
                            rhs=pT[:, tj, :],
                            start=(tj == 0),
                            stop=(tj == 7),
                        )
                    rrow = sm.tile([1, 512], F32, tag="rrow", bufs=4)
                    nc.vector.tensor_copy(out=rrow, in_=av_ps[DHD : DHD + 1, :])
                    nc.vector.reciprocal(out=rrow, in_=rrow)
                    rbc_ps = ps.tile([128, 512], F32, tag="pt", bufs=2)
                    nc.tensor.matmul(rbc_ps, lhsT=ones1, rhs=rrow, start=True, stop=True)
                    rbc_sb = sm.tile([128, 512], F32, tag="rbc", bufs=4)
                    nc.vector.tensor_copy(out=rbc_sb, in_=rbc_ps)
                    nc.vector.tensor_tensor(
                        out=aT[h // 2][hp : hp + 64, sh * 512 : (sh + 1) * 512],
                        in0=av_ps[0:DHD, :],
                        in1=rbc_sb[0:DHD, :],
                        op=mybir.AluOpType.mult,
                    )

            # ---------------- phase 3: WO + residual + LN1 ----------------
            x2_sb = [work.tile([128, D], F32, tag="v2k", bufs=9, name=f"x2_{i}") for i in range(N_ST)]
            x2T = [work.tile([128, S], F32, tag="big4k", bufs=14, name=f"x2T{i}") for i in range(3)]
            LCORR = float(D) / float(D - 1)

            def layer_norm(dst, src_ps, res_tiles, g_bc, b_bc, extra=None):
                # dst = LN(src_ps + residuals) * g + b   (src_ps in PSUM)
                xr = sm.tile([128, D], F32, tag="xr", bufs=4)
                nc.vector.tensor_add(xr, src_ps, res_tiles[0])
                for rt in res_tiles[1:]:
                    nc.vector.tensor_add(xr, xr, rt)
                stats = sm.tile([128, 6], F32, tag="lstats", bufs=4)
                nc.vector.bn_stats(out=stats, in_=xr)
                mv = sm.tile([128, 2], F32, tag="lmv", bufs=4)
                nc.vector.bn_aggr(out=mv, in_=stats)
                sd = sm.tile([128, 1], F32, tag="lsd", bufs=4)
                nc.scalar.activation(
                    out=sd, in_=mv[:, 1:2], func=AF.Sqrt, bias=eps_l, scale=LCORR
                )
                rstd = sm.tile([128, 1], F32, tag="lrstd", bufs=4)
                nc.vector.reciprocal(out=rstd, in_=sd)
                grstd = sm.tile([128, 1], F32, tag="lgr", bufs=4)
                nc.vector.tensor_mul(grstd, rstd, g_bc)
                nc.vector.tensor_scalar(
                    out=dst,
                    in0=xr,
                    scalar1=mv[:, 0:1],
                    scalar2=grstd,
                    op0=mybir.AluOpType.subtract,
                    op1=mybir.AluOpType.mult,
                )
                nc.vector.tensor_scalar_add(dst, dst, b_bc)

            for st in range(N_ST):
                x1_ps = ps.tile([128, D], F32, tag="e", bufs=6)
                for it in range(4):
                    nc.tensor.matmul(
                        x1_ps,
                        lhsT=aT[it][:, st * 128 : (st + 1) * 128],
                        rhs=wo_sb[it],
                        start=(it == 0),
                        stop=(it == 3),
                    )
                layer_norm(x2_sb[st], x1_ps, [x_sb[:, st, :]], g1_bc, b1_bc)
                # transpose x2[st] -> x2T chunks
                xt_ps = ps.tile([128, 4, 128], F32, tag="pt", bufs=2)
                for jc, (j0, jn) in enumerate(J_CHUNKS):
                    nc.tensor.transpose(
                        xt_ps[:jn, jc, :], x2_sb[st][:, j0 : j0 + jn], ident
                    )
                for jc, (j0, jn) in enumerate(J_CHUNKS):
                    nc.vector.tensor_copy(
                        out=x2T[jc][:jn, st * 128 : (st + 1) * 128],
                        in_=xt_ps[:jn, jc, :],
                    )

            # ---------------- phase 4: FFN + LN2 ----------------
            h1T = [work.tile([128, S], F32, tag="big4k", bufs=14, name=f"h1T{i}") for i in range(10)]
            for mt, (m0, msz) in enumerate(M_CHUNKS):
                for sh in range(N_SH):
                    h1_ps = ps.tile([128, 512], F32, tag="e", bufs=6)
                    for jc, (j0, jn) in enumerate(J_CHUNKS):
                        nc.tensor.matmul(
                            h1_ps[:msz, :],
                            lhsT=w1_sb[jc][:jn, m0 : m0 + msz],
                            rhs=x2T[jc][:jn, sh * 512 : (sh + 1) * 512],
                            start=(jc == 0),
                            stop=(jc == 2),
                        )
                    nc.scalar.activation(
                        out=h1T[mt][:msz, sh * 512 : (sh + 1) * 512],
                        in_=h1_ps[:msz, :],
                        func=AF.Relu,
                        bias=fb1_sb[mt][:msz, :],
                        scale=1.0,
                    )
            if dbg:
                for i in range(N_ST):
                    nc.sync.dma_start(out=dx2[i * 128 : (i + 1) * 128, :], in_=x2_sb[i])
                for mt, (m0, msz) in enumerate(M_CHUNKS):
                    nc.sync.dma_start(out=dh1[m0 : m0 + msz, :], in_=h1T[mt][:msz, :])
            for st in range(N_ST):
                h2_ps = ps.tile([128, D], F32, tag="e", bufs=6)
                for mt, (m0, msz) in enumerate(M_CHUNKS):
                    nc.tensor.matmul(
                        h2_ps,
                        lhsT=h1T[mt][:msz, st * 128 : (st + 1) * 128],
                        rhs=w2_sb[mt][:msz, :],
                        start=(mt == 0),
                        stop=(mt == 9),
                    )
                o_sb = sm.tile([128, D], F32, tag="o", bufs=4)
                layer_norm(o_sb, h2_ps, [fb2_bc, x2_sb[st]], g2_bc, b2_bc)
                nc.sync.dma_start(out=outd[st * 128 : (st + 1) * 128, :], in_=o_sb)

    nc.compile()
    return nc


def _get_nc():
    if "nc" not in _cache:
        _cache["nc"] = _build_nc()
    return _cache["nc"]


def kernel(x, WQ, WK, WV, WO, W1, b1, W2, b2, gamma_a, beta_a,
           gamma1, beta1, gamma2, beta2):
    global _last_results
    f = np.float32
    x = np.asarray(x, f)

    def perm(W):
        # head h -> contiguous rows [h*64, (h+1)*64)
        return np.asarray(W, f).reshape(DHD, H, D).transpose(1, 0, 2).reshape(DH, D)

    wq_t = np.ascontiguousarray(perm(WQ).T)
    wk_t = np.ascontiguousarray(perm(WK).T)
    wv_t = np.ascontiguousarray(perm(WV).T)
    wo = np.ascontiguousarray(np.asarray(WO, f))
    w1 = np.ascontiguousarray(np.asarray(W1, f))
    w2 = np.ascontiguousarray(np.asarray(W2, f))
    fb1 = np.zeros((1280, 1), f)
    fb1[:DF, 0] = np.asarray(b1, f)
    fb2 = np.ascontiguousarray(np.asarray(b2, f))
    # beta_a drops out of softmax (per-row constant shift); the 1/sqrt(D)
    # score scale cancels inside the score LayerNorm: softmax(g*LN(e/sqrt(D)))
    # == softmax(g/sqrt(var(e) + D*eps) * e), so gamma is used unscaled and
    # D*eps replaces eps on-device.
    ga = np.ascontiguousarray(np.asarray(gamma_a, f).reshape(H, 1))
    ln = np.array(
        [np.asarray(gamma1, f), np.asarray(beta1, f),
         np.asarray(gamma2, f), np.asarray(beta2, f)], f
    ).reshape(4, 1)

    shared = {"wq": wq_t, "wk": wk_t, "wv": wv_t, "wo": wo, "w1": w1, "w2": w2,
              "fb1": fb1, "fb2": fb2, "ga": ga, "ln": ln}
    in_maps = []
    for b in range(B):
        xb = np.ascontiguousarray(x[b])
        in_maps.append({"x": xb, "xt": np.ascontiguousarray(xb.T), **shared})

    nc = _get_nc()
    res = run_bass_kernel_spmd(nc, in_maps, core_ids=list(range(NCORES)), trace=TRACE)
    _last_results = res
    return np.stack([res.results[b]["out"] for b in range(B)], axis=0)


# revision 13
# speedup vs baseline: 2.6278x; 1.4209x over previous
"""Trainium2 Bass kernel: 8-head transformer encoder layer (B=8, S=1024,
D=300, Dh=512, H=8), data-parallel over batch across 8 NeuronCores.

Per core (one batch element):
  qT/kT = Wp @ x^T  (heads contiguous via host-side weight-row permute)
  v     = x @ Wp^T
  per head: e = q k^T (PSUM) -> bn_stats var -> c = gamma*sqrt(D)/sd
            p = exp(c*e) (ACT, accum row-sum r) -> p *= 1/r (GPSIMD)
            pT via PE transpose -> heads^T = v^T-chunks @ pT (PSUM acc)
  x1 = a @ WO ; x2 = LN(x1 + x) ; x2T via PE transpose
  h1T = relu(W1^T-form @ x2T + b1) ; h2 = h1T-chunks @ W2
  out = LN(h2 + b2 + x2)
"""

import math

import numpy as np

import concourse.bass as bass
import concourse.tile as tile
from concourse import bacc, mybir
from concourse.bass_utils import run_bass_kernel_spmd
from concourse.masks import make_identity

F32 = mybir.dt.float32
USE_F32R = True
FR = mybir.dt.float32r if USE_F32R else F32
AF = mybir.ActivationFunctionType

B, S, D, DH, H, DHD = 8, 1024, 300, 512, 8, 64
DF = 4 * D  # 1200
EPS = 1e-8
NCORES = 8

J_CHUNKS = [(0, 128), (128, 128), (256, 44)]  # D=300 partition chunks
M_CHUNKS = [(i * 128, min(128, DF - i * 128)) for i in range(10)]  # DF=1200
N_ST = S // 128  # 8 s-tiles
N_SH = S // 512  # 2 s-halves

TRACE = False
_cache = {}
_last_results = None


def _build_nc(dbg=False):
    nc = bacc.Bacc("TRN2", debug=False)

    xd = nc.dram_tensor("x", [S, D], F32, kind="ExternalInput").ap()
    xtd = nc.dram_tensor("xt", [D, S], F32, kind="ExternalInput").ap()
    wqd = nc.dram_tensor("wq", [D, DH], F32, kind="ExternalInput").ap()
    wkd = nc.dram_tensor("wk", [D, DH], F32, kind="ExternalInput").ap()
    wvd = nc.dram_tensor("wv", [D, DH], F32, kind="ExternalInput").ap()
    wod = nc.dram_tensor("wo", [DH, D], F32, kind="ExternalInput").ap()
    w1d = nc.dram_tensor("w1", [D, DF], F32, kind="ExternalInput").ap()
    w2d = nc.dram_tensor("w2", [DF, D], F32, kind="ExternalInput").ap()
    fb1d = nc.dram_tensor("fb1", [1280, 1], F32, kind="ExternalInput").ap()
    fb2d = nc.dram_tensor("fb2", [D], F32, kind="ExternalInput").ap()
    gad = nc.dram_tensor("ga", [H, 1], F32, kind="ExternalInput").ap()
    lnd = nc.dram_tensor("ln", [4, 1], F32, kind="ExternalInput").ap()
    outd = nc.dram_tensor("out", [S, D], F32, kind="ExternalOutput").ap()
    if dbg:
        dqT = nc.dram_tensor("dqT", [DH, S], F32, kind="ExternalOutput").ap()
        dkT = nc.dram_tensor("dkT", [DH, S], F32, kind="ExternalOutput").ap()
        dv = nc.dram_tensor("dv", [S, DH], F32, kind="ExternalOutput").ap()
        dp0 = nc.dram_tensor("dp0", [S, S], F32, kind="ExternalOutput").ap()
        daT = nc.dram_tensor("daT", [DH, S], F32, kind="ExternalOutput").ap()
        dx2 = nc.dram_tensor("dx2", [S, D], F32, kind="ExternalOutput").ap()
        dh1 = nc.dram_tensor("dh1", [DF, S], F32, kind="ExternalOutput").ap()
        dr = nc.dram_tensor("dr", [H, S], F32, kind="ExternalOutput").ap()
        dc = nc.dram_tensor("dc", [H, S], F32, kind="ExternalOutput").ap()
        dmv = nc.dram_tensor("dmv", [H, S, 2], F32, kind="ExternalOutput").ap()

    def bcr(ap):
        return ap.bitcast(FR) if USE_F32R else ap

    with tile.TileContext(nc) as tc:
        with (
            tc.tile_pool(name="wts", bufs=1) as wts,
            tc.tile_pool(name="work", bufs=1) as work,
            tc.tile_pool(name="sm", bufs=8) as sm,
            tc.tile_pool(name="ps", bufs=1, space="PSUM") as ps,
        ):
            # ---------------- constant / weight loads ----------------
            ident = wts.tile([128, 128], F32, tag="ident")
            make_identity(nc, ident)
            identr = wts.tile([128, 128], FR, tag="identr")
            nc.vector.tensor_copy(out=identr, in_=ident)

            ones1 = wts.tile([1, 128], F32, tag="ones1")
            nc.vector.memset(ones1, 1.0)
            dummy = wts.tile([128, 1], F32, tag="dummy")
            nc.vector.memset(dummy, 1.0)
            dsink = wts.tile([128, 1], F32, tag="dsink")

            eps_a = wts.tile([128, 1], F32, tag="eps_a")  # D*EPS (score LN)
            nc.vector.memset(eps_a, D * EPS)
            eps_l = wts.tile([128, 1], F32, tag="eps_l")  # EPS (x LNs)
            nc.vector.memset(eps_l, EPS)

            def bcast_load(src_ap, shape, tag):
                t = wts.tile(shape, F32, tag=tag)
                nc.sync.dma_start(out=t, in_=src_ap.to_broadcast(shape))
                return t

            ga_bc = [bcast_load(gad[h : h + 1, :], [128, 1], f"ga{h}") for h in range(H)]
            g1_bc = bcast_load(lnd[0:1, :], [128, 1], "g1")
            b1_bc = bcast_load(lnd[1:2, :], [128, 1], "b1")
            g2_bc = bcast_load(lnd[2:3, :], [128, 1], "g2")
            b2_bc = bcast_load(lnd[3:4, :], [128, 1], "b2")
            fb2_bc = wts.tile([128, D], F32, tag="fb2")
            nc.sync.dma_start(
                out=fb2_bc,
                in_=bass.AP(tensor=fb2d.tensor, offset=fb2d.offset, ap=[[0, 128]] + list(fb2d.ap)),
            )
            fb1_sb = []
            for mt, (m0, msz) in enumerate(M_CHUNKS):
                t = wts.tile([128, 1], F32, tag=f"fb1_{mt}")
                nc.sync.dma_start(out=t[:msz, :], in_=fb1d[m0 : m0 + msz, :])
                fb1_sb.append(t)

            # x natural: [128, 8, 300] (partition = s % 128)
            x_sb = wts.tile([128, N_ST, D], F32, tag="x")
            nc.sync.dma_start(out=x_sb, in_=xd.rearrange("(n p) d -> p n d", p=128))

            def chunked_load(src, width, tag):
                tiles = []
                for jc, (j0, jn) in enumerate(J_CHUNKS):
                    t = wts.tile([128, width], FR, tag=f"{tag}{jc}")
                    nc.sync.dma_start(out=t[:jn, :], in_=bcr(src[j0 : j0 + jn, :]))
                    tiles.append(t)
                return tiles

            xt_sb = chunked_load(xtd, S, "xt")    # [300, 1024] in 3 chunks
            wq_sb = chunked_load(wqd, DH, "wq")   # [300, 512]
            wk_sb = chunked_load(wkd, DH, "wk")
            wv_sb = chunked_load(wvd, DH, "wv")
            w1_sb = chunked_load(w1d, DF, "w1")   # [300, 1200]

            wo_sb = []
            for it in range(4):
                t = wts.tile([128, D], FR, tag=f"wo{it}")
                nc.sync.dma_start(out=t, in_=bcr(wod[it * 128 : (it + 1) * 128, :]))
                wo_sb.append(t)
            w2_sb = []
            for mt, (m0, msz) in enumerate(M_CHUNKS):
                t = wts.tile([128, D], FR, tag=f"w2_{mt}")
                nc.sync.dma_start(out=t[:msz, :], in_=bcr(w2d[m0 : m0 + msz, :]))
                w2_sb.append(t)

            # ---------------- phase 1: projections ----------------
            # qT/kT [Dh, S] as 4 x [128, 1024];  v [S, Dh] as 8 x [128, 512]
            qT = [work.tile([128, S], FR, tag="big4k", bufs=14, name=f"qT{i}") for i in range(4)]
            kT = [work.tile([128, S], FR, tag="big4k", bufs=14, name=f"kT{i}") for i in range(4)]
            v_sb = [work.tile([128, H, DHD + 1], FR, tag="v2k", bufs=9, name=f"v{i}") for i in range(N_ST)]

            for dst, w in ((qT, wq_sb), (kT, wk_sb)):
                for dt in range(4):
                    for sh in range(N_SH):
                        pp = ps.tile([128, 512], F32, tag="e", bufs=6)
                        for jc, (j0, jn) in enumerate(J_CHUNKS):
                            nc.tensor.matmul(
                                pp,
                                lhsT=w[jc][:jn, dt * 128 : (dt + 1) * 128],
                                rhs=xt_sb[jc][:jn, sh * 512 : (sh + 1) * 512],
                                start=(jc == 0),
                                stop=(jc == 2),
                            )
                        nc.vector.tensor_copy(out=dst[dt][:, sh * 512 : (sh + 1) * 512], in_=pp)
            for st in range(N_ST):
                pp = ps.tile([128, 512], F32, tag="e", bufs=6)
                for jc, (j0, jn) in enumerate(J_CHUNKS):
                    nc.tensor.matmul(
                        pp,
                        lhsT=xt_sb[jc][:jn, st * 128 : (st + 1) * 128],
                        rhs=wv_sb[jc][:jn, :],
                        start=(jc == 0),
                        stop=(jc == 2),
                    )
                nc.vector.tensor_copy(
                    out=v_sb[st][:, :, 0:DHD],
                    in_=pp.rearrange("p (h d) -> p h d", h=H),
                )
                nc.vector.memset(v_sb[st][:, :, DHD : DHD + 1].bitcast(F32), 1.0)

            if dbg:
                for i in range(4):
                    nc.sync.dma_start(out=dqT[i * 128 : (i + 1) * 128, :], in_=qT[i])
                    nc.sync.dma_start(out=dkT[i * 128 : (i + 1) * 128, :], in_=kT[i])
                for i in range(N_ST):
                    nc.sync.dma_start(out=dv[i * 128 : (i + 1) * 128, :], in_=v_sb[i][:, :, 0:DHD])

            # ---------------- phase 2: attention ----------------
            aT = [work.tile([128, S], FR, tag="big4k", bufs=14, name=f"aT{i}") for i in range(4)]
            SCORR = float(S) / float(S - 1)

            for h in range(H):
                qt_t, hp = qT[h // 2], (h % 2) * 64
                kt_t = kT[h // 2]
                for sh in range(N_SH):
                    pT = work.tile([128, 8, 512], FR, tag="pt16k", bufs=2)
                    for pair in range(2):
                        e_pair = []
                        mv2 = sm.tile([128, 2, 2], F32, tag="mv", bufs=4)
                        for i in range(2):
                            st = sh * 4 + pair * 2 + i
                            eh0 = ps.tile([128, 512], F32, tag="e", bufs=6, name="eh0")
                            eh1 = ps.tile([128, 512], F32, tag="e", bufs=6, name="eh1")
                            e_pair.append((i, st, eh0, eh1))
                            stats = sm.tile([128, 2, 6], F32, tag="stats", bufs=4)
                            for th, eh in ((0, eh0), (1, eh1)):
                                nc.tensor.matmul(
                                    eh,
                                    lhsT=qt_t[hp : hp + 64, st * 128 : (st + 1) * 128],
                                    rhs=kt_t[hp : hp + 64, th * 512 : (th + 1) * 512],
                                    start=True,
                                    stop=True,
                                )
                                nc.vector.bn_stats(out=stats[:, th, :], in_=eh)
                            nc.vector.bn_aggr(out=mv2[:, i, :], in_=stats)
                        # sd = sqrt(var*S/(S-1) + D*eps) batched over the pair
                        c2 = sm.tile([128, 2], F32, tag="c", bufs=4)
                        nc.scalar.activation(
                            out=c2, in_=mv2[:, :, 1], func=AF.Sqrt, bias=eps_a, scale=SCORR
                        )
                        nc.scalar.activation(
                            out=dsink, in_=dummy, func=AF.Exp, bias=0.0, scale=1.0
                        )
                        nc.vector.reciprocal(out=c2, in_=c2)
                        nc.vector.tensor_scalar_mul(c2, c2, ga_bc[h])
                        for i, st, eh0, eh1 in e_pair:
                            st4 = pair * 2 + i
                            p_sb = work.tile([128, S], FR, tag="big4k", bufs=14)
                            for th, eh in ((0, eh0), (1, eh1)):
                                nc.scalar.activation(
                                    out=p_sb[:, th * 512 : (th + 1) * 512],
                                    in_=eh, func=AF.Exp, bias=0.0,
                                    scale=c2[:, i : i + 1],
                                )
                            if i == 1:
                                nc.scalar.activation(
                                    out=dsink, in_=dummy, func=AF.Sqrt,
                                    bias=0.0, scale=1.0,
                                )
                            for half in range(2):
                                pt_ps = ps.tile([128, 4, 128], FR, tag="pt", bufs=2)
                                for k in range(4):
                                    tj = half * 4 + k
                                    nc.tensor.transpose(
                                        pt_ps[:, k, :],
                                        p_sb[:, tj * 128 : (tj + 1) * 128],
                                        identr,
                                    )
                                nc.vector.tensor_copy(
                                    out=pT[:, half * 4 : half * 4 + 4,
                                           st4 * 128 : (st4 + 1) * 128],
                                    in_=pt_ps,
                                )
                    # AV for this half: [65, 512]; row 64 = softmax denominator
                    av_ps = ps.tile([DHD + 1, 512], F32, tag="pt", bufs=2)
                    for tj in range(8):
                        nc.tensor.matmul(
                            av_ps,
                            lhsT=v_sb[tj][:, h, :],
                            rhs=pT[:, tj, :],
                            start=(tj == 0),
                            stop=(tj == 7),
                        )
                    rrow = sm.tile([1, 512], F32, tag="rrow", bufs=4)
                    nc.vector.tensor_copy(out=rrow, in_=av_ps[DHD : DHD + 1, :])
                    nc.vector.reciprocal(out=rrow, in_=rrow)
                    rbc_ps = ps.tile([128, 512], F32, tag="pt", bufs=2)
                    nc.tensor.matmul(rbc_ps, lhsT=ones1, rhs=rrow, start=True, stop=True)
                    rbc_sb = sm.tile([128, 512], F32, tag="rbc", bufs=4)
                    nc.vector.tensor_copy(out=rbc_sb, in_=rbc_ps)
                    nc.vector.tensor_tensor(
                        out=aT[h // 2][hp : hp + 64, sh * 512 : (sh + 1) * 512],
                        in0=av_ps[0:DHD, :],
                        in1=rbc_sb[0:DHD, :],
                        op=mybir.AluOpType.mult,
                    )

            # ---------------- phase 3: WO + residual + LN1 ----------------
            x2_sb = [work.tile([128, D], F32, tag="v2k", bufs=9, name=f"x2_{i}") for i in range(N_ST)]
            x2T = [work.tile([128, S], FR, tag="big4k", bufs=14, name=f"x2T{i}") for i in range(3)]
            LCORR = float(D) / float(D - 1)

            def layer_norm(dst, src_ps, res_tiles, g_bc, b_bc, extra=None):
                # dst = LN(src_ps + residuals) * g + b   (src_ps in PSUM)
                xr = sm.tile([128, D], F32, tag="xr", bufs=4)
                nc.vector.tensor_add(xr, src_ps, res_tiles[0])
                for rt in res_tiles[1:]:
                    nc.vector.tensor_add(xr, xr, rt)
                stats = sm.tile([128, 6], F32, tag="lstats", bufs=4)
                nc.vector.bn_stats(out=stats, in_=xr)
                mv = sm.tile([128, 2], F32, tag="lmv", bufs=4)
                nc.vector.bn_aggr(out=mv, in_=stats)
                sd = sm.tile([128, 1], F32, tag="lsd", bufs=4)
                nc.scalar.activation(
                    out=sd, in_=mv[:, 1:2], func=AF.Sqrt, bias=eps_l, scale=LCORR
                )
                rstd = sm.tile([128, 1], F32, tag="lrstd", bufs=4)
                nc.vector.reciprocal(out=rstd, in_=sd)
                grstd = sm.tile([128, 1], F32, tag="lgr", bufs=4)
                nc.vector.tensor_mul(grstd, rstd, g_bc)
                nc.vector.tensor_scalar(
                    out=dst,
                    in0=xr,
                    scalar1=mv[:, 0:1],
                    scalar2=grstd,
                    op0=mybir.AluOpType.subtract,
                    op1=mybir.AluOpType.mult,
                )
                nc.vector.tensor_scalar_add(dst, dst, b_bc)

            for st in range(N_ST):
                x1_ps = ps.tile([128, D], F32, tag="e", bufs=6)
                for it in range(4):
                    nc.tensor.matmul(
                        x1_ps,
                        lhsT=aT[it][:, st * 128 : (st + 1) * 128],
                        rhs=wo_sb[it],
                        start=(it == 0),
                        stop=(it == 3),
                    )
                layer_norm(x2_sb[st], x1_ps, [x_sb[:, st, :]], g1_bc, b1_bc)
                # transpose x2[st] -> x2T chunks
                xt_ps = ps.tile([128, 4, 128], F32, tag="pt", bufs=2)
                for jc, (j0, jn) in enumerate(J_CHUNKS):
                    nc.tensor.transpose(
                        xt_ps[:jn, jc, :], x2_sb[st][:, j0 : j0 + jn], ident
                    )
                for jc, (j0, jn) in enumerate(J_CHUNKS):
                    nc.vector.tensor_copy(
                        out=x2T[jc][:jn, st * 128 : (st + 1) * 128],
                        in_=xt_ps[:jn, jc, :],
                    )

            # ---------------- phase 4: FFN + LN2 ----------------
            h1T = [work.tile([128, S], FR, tag="big4k", bufs=14, name=f"h1T{i}") for i in range(10)]
            for mt, (m0, msz) in enumerate(M_CHUNKS):
                for sh in range(N_SH):
                    h1_ps = ps.tile([128, 512], F32, tag="e", bufs=6)
                    for jc, (j0, jn) in enumerate(J_CHUNKS):
                        nc.tensor.matmul(
                            h1_ps[:msz, :],
                            lhsT=w1_sb[jc][:jn, m0 : m0 + msz],
                            rhs=x2T[jc][:jn, sh * 512 : (sh + 1) * 512],
                            start=(jc == 0),
                            stop=(jc == 2),
                        )
                    nc.scalar.activation(
                        out=h1T[mt][:msz, sh * 512 : (sh + 1) * 512],
                        in_=h1_ps[:msz, :],
                        func=AF.Relu,
                        bias=fb1_sb[mt][:msz, :],
                        scale=1.0,
                    )
            if dbg:
                for i in range(N_ST):
                    nc.sync.dma_start(out=dx2[i * 128 : (i + 1) * 128, :], in_=x2_sb[i])
                for mt, (m0, msz) in enumerate(M_CHUNKS):
                    nc.sync.dma_start(out=dh1[m0 : m0 + msz, :], in_=h1T[mt][:msz, :])
            for st in range(N_ST):
                h2_ps = ps.tile([128, D], F32, tag="e", bufs=6)
                for mt, (m0, msz) in enumerate(M_CHUNKS):
                    nc.tensor.matmul(
                        h2_ps,
                        lhsT=h1T[mt][:msz, st * 128 : (st + 1) * 128],
                        rhs=w2_sb[mt][:msz, :],
                        start=(mt == 0),
                        stop=(mt == 9),
                    )
                o_sb = sm.tile([128, D], F32, tag="o", bufs=4)
                layer_norm(o_sb, h2_ps, [fb2_bc, x2_sb[st]], g2_bc, b2_bc)
                nc.sync.dma_start(out=outd[st * 128 : (st + 1) * 128, :], in_=o_sb)

    nc.compile()
    return nc


def _get_nc():
    if "nc" not in _cache:
        _cache["nc"] = _build_nc()
    return _cache["nc"]


def kernel(x, WQ, WK, WV, WO, W1, b1, W2, b2, gamma_a, beta_a,
           gamma1, beta1, gamma2, beta2):
    global _last_results
    f = np.float32
    x = np.asarray(x, f)

    def perm(W):
        # head h -> contiguous rows [h*64, (h+1)*64)
        return np.asarray(W, f).reshape(DHD, H, D).transpose(1, 0, 2).reshape(DH, D)

    wq_t = np.ascontiguousarray(perm(WQ).T)
    wk_t = np.ascontiguousarray(perm(WK).T)
    wv_t = np.ascontiguousarray(perm(WV).T)
    wo = np.ascontiguousarray(np.asarray(WO, f))
    w1 = np.ascontiguousarray(np.asarray(W1, f))
    w2 = np.ascontiguousarray(np.asarray(W2, f))
    fb1 = np.zeros((1280, 1), f)
    fb1[:DF, 0] = np.asarray(b1, f)
    fb2 = np.ascontiguousarray(np.asarray(b2, f))
    # beta_a drops out of softmax (per-row constant shift); the 1/sqrt(D)
    # score scale cancels inside the score LayerNorm: softmax(g*LN(e/sqrt(D)))
    # == softmax(g/sqrt(var(e) + D*eps) * e), so gamma is used unscaled and
    # D*eps replaces eps on-device.
    ga = np.ascontiguousarray(np.asarray(gamma_a, f).reshape(H, 1))
    ln = np.array(
        [np.asarray(gamma1, f), np.asarray(beta1, f),
         np.asarray(gamma2, f), np.asarray(beta2, f)], f
    ).reshape(4, 1)

    shared = {"wq": wq_t, "wk": wk_t, "wv": wv_t, "wo": wo, "w1": w1, "w2": w2,
              "fb1": fb1, "fb2": fb2, "ga": ga, "ln": ln}
    in_maps = []
    for b in range(B):
        xb = np.ascontiguousarray(x[b])
        in_maps.append({"x": xb, "xt": np.ascontiguousarray(xb.T), **shared})

    nc = _get_nc()
    res = run_bass_kernel_spmd(nc, in_maps, core_ids=list(range(NCORES)), trace=TRACE)
    _last_results = res
    return np.stack([res.results[b]["out"] for b in range(B)], axis=0)
